# revision 4
# baseline (speedup 1.0000x reference)
"""Sharded DenseGNN Bass kernel for 8 TRN2 NeuronCores.

Design:
  - Nodes partitioned contiguously across 8 cores (6250/core), degree-sorted
    per core, padded to 49 tiles of 128 (22 dummy slots).
  - Per GCN layer: each core computes hw = (h @ W') * dinv for its local
    slab, all-gathers slabs into a DRAM table [50176, 64], then dma_gathers
    per-edge source rows (slot-padded per node, A/B window split for int16
    indices) and segment-reduces on the vector engine.
  - BN folded into W' and a per-feature bias; symmetric normalization folded
    into per-node dinv pre/post scaling.
  - Per-graph mean pooling via one-hot matmul, cross-core AllReduce of the
    tiny [8, 65] pool partials, decoder computed redundantly on every core.

Structural preprocessing (index manipulation only) happens on the host.
"""
import sys
import types

sys.path.insert(0, "/opt/trn_rl_repo")
if "antenv.axon_hooks" not in sys.modules:
    try:
        import antenv  # noqa: F401
        _m = types.ModuleType("antenv.axon_hooks")
        _m.get_axon_ntff_profile_hook = lambda: None
        sys.modules["antenv.axon_hooks"] = _m
    except ImportError:
        pass

import numpy as np

from concourse import bacc, bass, mybir, tile
from concourse.masks import make_identity

F32 = mybir.dt.float32
I32 = mybir.dt.int32
I16 = mybir.dt.int16
NG = 8
BN_EPS = 1e-5


class Config:
    def __init__(self, n_nodes, n_cores=8, max_idxs_per_call=12288):
        self.N = n_nodes
        self.C = n_cores
        self.NPC = n_nodes // n_cores
        self.TILE = 128
        self.TPC = (self.NPC + 127) // 128
        self.SPC = self.TPC * 128
        self.TBL = self.C * self.SPC
        self.WINB = max(0, self.TBL - 32767)
        self.WINA_ROWS = min(self.TBL, 32768)
        self.WINB_ROWS = self.TBL - self.WINB
        # pads point at dummy slots (always-zero rows are not needed; dummy
        # hw rows are explicitly zeroed before each all-gather)
        assert self.SPC > self.NPC, "need dummy slots for pad targets"
        self.PAD_A_GPOS = self.NPC                      # core 0 first dummy
        self.PAD_B_GPOS = (self.C - 1) * self.SPC + self.NPC  # last core dummy
        assert self.PAD_A_GPOS < self.WINA_ROWS
        assert self.PAD_B_GPOS >= self.WINB
        self.PAD_A = self.PAD_A_GPOS
        self.PAD_B = self.PAD_B_GPOS - self.WINB
        assert self.PAD_B < 32768
        self.MAX_IDXS = max_idxs_per_call


CFG_FULL = Config(50000)


def preprocess(cfg, edge_index, batch):
    """Build the SPMD-uniform schedule + per-core index data."""
    src_g = np.asarray(edge_index[0], dtype=np.int64)
    dst_g = np.asarray(edge_index[1], dtype=np.int64)
    batch = np.asarray(batch, dtype=np.int64)
    N, C, NPC, SPC, TPC = cfg.N, cfg.C, cfg.NPC, cfg.SPC, cfg.TPC

    deg = np.bincount(dst_g, minlength=N).astype(np.int64) + 1
    # global degree sort, dealt into cores in blocks of 128 so tile k has a
    # near-identical degree band on every core (minimizes cross-core-max pad)
    order_g = np.argsort(-deg, kind="stable")
    perm = np.empty(N, dtype=np.int64)
    gpos = np.empty(N, dtype=np.int64)
    ntile_g = N // (C * 128)           # full global tile groups
    pos = 0
    for k in range(ntile_g):
        blk = order_g[pos:pos + C * 128]
        for c in range(C):
            nodes = blk[c * 128:(c + 1) * 128]
            perm[c * NPC + k * 128:c * NPC + (k + 1) * 128] = nodes
            gpos[nodes] = c * SPC + k * 128 + np.arange(128)
        pos += C * 128
    rem = order_g[pos:]                # leftover < C*128 nodes -> round robin
    for c in range(C):
        nodes = rem[c::C]
        nloc = ntile_g * 128
        perm[c * NPC + nloc:c * NPC + nloc + nodes.size] = nodes
        gpos[nodes] = c * SPC + nloc + np.arange(nodes.size)

    all_src = np.concatenate([src_g, np.arange(N)])
    all_dst = np.concatenate([dst_g, np.arange(N)])
    sgp = gpos[all_src]
    owner = gpos[all_dst] // SPC

    # classify for window split: 0=mustA, 1=flex, 2=mustB
    cls = np.ones(all_src.shape, dtype=np.int8)
    cls[sgp < cfg.WINB] = 0
    cls[sgp > 32767] = 2

    per_core = []
    dA_all = np.zeros((C, SPC), dtype=np.int64)
    dB_all = np.zeros((C, SPC), dtype=np.int64)
    for c in range(C):
        m = owner == c
        e_sgp = sgp[m]
        e_cls = cls[m]
        dst_slot = gpos[all_dst[m]] - c * SPC
        order = np.lexsort((e_cls, dst_slot))
        e_sgp = e_sgp[order]
        e_cls = e_cls[order]
        dst_slot = dst_slot[order]
        counts = np.bincount(dst_slot, minlength=SPC)
        offs = np.concatenate([[0], np.cumsum(counts)])
        nA_must = np.bincount(dst_slot[e_cls == 0], minlength=SPC)
        nflex = np.bincount(dst_slot[e_cls == 1], minlength=SPC)
        need_a = np.clip((counts + 1) // 2 - nA_must, 0, nflex)
        dA = nA_must + need_a
        dB = counts - dA
        dA_all[c] = dA
        dB_all[c] = dB
        per_core.append(dict(e_sgp=e_sgp, offs=offs, dA=dA, dB=dB))

    # cross-core uniform per-tile slot widths
    DA = np.maximum(dA_all.reshape(C, TPC, 128).max(axis=(0, 2)), 1)
    DB = np.maximum(dB_all.reshape(C, TPC, 128).max(axis=(0, 2)), 1)

    # per-core padded slot tables [128, sum(D)] int32 (window-local indices)
    SA, SB = int(DA.sum()), int(DB.sum())
    offA = np.concatenate([[0], np.cumsum(DA)]).astype(np.int64)
    offB = np.concatenate([[0], np.cumsum(DB)]).astype(np.int64)
    intA = np.full((C, 128, SA), cfg.PAD_A, dtype=np.int32)
    intB = np.full((C, 128, SB), cfg.PAD_B, dtype=np.int32)
    for c in range(C):
        pc = per_core[c]
        e_sgp, offs, dA, dB = pc["e_sgp"], pc["offs"], pc["dA"], pc["dB"]
        # positions of each edge within its node's run
        node_of_edge = np.repeat(np.arange(SPC), np.diff(offs))
        rank = np.arange(e_sgp.size) - offs[node_of_edge]
        isA = rank < dA[node_of_edge]
        t_of_node = node_of_edge // 128
        p_of_node = node_of_edge % 128
        # A entries
        ra = rank[isA]
        intA[c, p_of_node[isA], offA[t_of_node[isA]] + ra] = e_sgp[isA]
        rb = rank[~isA] - dA[node_of_edge[~isA]]
        intB[c, p_of_node[~isA], offB[t_of_node[~isA]] + rb] = (
            e_sgp[~isA] - cfg.WINB)

    # gather-call schedule (uniform): batch consecutive tiles per window
    calls = []
    idx_off = 0
    for win, D, off in ((0, DA, offA), (1, DB, offB)):
        t = 0
        while t < TPC:
            t0, nidx, chunks = t, 0, 0
            while t < TPC:
                d = int(D[t])
                if nidx + d * 128 > cfg.MAX_IDXS and t > t0:
                    break
                chunks += d
                nidx += d * 128
                t += 1
            calls.append(dict(win=win, t0=t0, ntiles=t - t0, idx_off=idx_off,
                              nidx=nidx, chunks=chunks,
                              tile_D=[int(D[tt]) for tt in range(t0, t)]))
            idx_off += nidx
    total_idx = idx_off
    assert total_idx % 128 == 0
    T16 = total_idx // 16

    # per-core wrapped int16 idx lists
    idx16 = np.empty((C, 128, T16), dtype=np.int16)
    for c in range(C):
        flat = np.empty(total_idx, dtype=np.int16)
        for call in calls:
            win = call["win"]
            tbl = intA[c] if win == 0 else intB[c]
            off = offA if win == 0 else offB
            pos = call["idx_off"]
            for k, tt in enumerate(range(call["t0"], call["t0"] + call["ntiles"])):
                d = call["tile_D"][k]
                blk = tbl[:, off[tt]:off[tt] + d]      # [128, d]
                flat[pos:pos + d * 128] = blk.T.reshape(-1).astype(np.int16)
                pos += d * 128
            assert pos == call["idx_off"] + call["nidx"]
        wrapped = flat.reshape(-1, 16).T               # [16, T16]
        idx16[c] = np.tile(wrapped, (8, 1))

    sched = dict(cfg=cfg, DA=DA, DB=DB, SA=SA, SB=SB,
                 offA=offA, offB=offB, calls=calls, T16=T16,
                 perm=perm, gpos=gpos)
    data = dict(idx16=idx16, intA=intA, intB=intB)
    return sched, data


def per_core_inputs(cfg, sched, data, inputs):
    """Build in_maps for run_bass_kernel_spmd."""
    x = np.asarray(inputs["x"], dtype=np.float32)
    batch = np.asarray(inputs["batch"], dtype=np.int64)
    perm = sched["perm"]
    C, NPC, SPC, TPC = cfg.C, cfg.NPC, cfg.SPC, cfg.TPC
    FIN = x.shape[1]

    common = {}
    for i in range(3):
        for k in ("W", ):
            common[f"W{i}"] = np.ascontiguousarray(inputs[f"W{i}"], dtype=np.float32)
        for k in ("b", "g", "beta", "rm", "rv"):
            common[f"{k}{i}"] = np.ascontiguousarray(
                np.asarray(inputs[f"{k}{i}"], dtype=np.float32).reshape(-1, 1))
    common["enc_w0"] = np.ascontiguousarray(inputs["enc_w0"], dtype=np.float32)
    common["enc_w1"] = np.ascontiguousarray(inputs["enc_w1"], dtype=np.float32)
    common["dec_w0"] = np.ascontiguousarray(inputs["dec_w0"], dtype=np.float32)
    common["dec_w1"] = np.ascontiguousarray(inputs["dec_w1"], dtype=np.float32)
    for k in ("enc_b0", "enc_b1", "dec_b0", "dec_b1"):
        common[k] = np.ascontiguousarray(
            np.asarray(inputs[k], dtype=np.float32).reshape(-1, 1))

    in_maps = []
    for c in range(C):
        nodes = perm[c * NPC:(c + 1) * NPC]
        xp = np.zeros((128, TPC * FIN), dtype=np.float32)
        xv = x[nodes]                                  # [NPC, FIN]
        xv = np.concatenate(
            [xv, np.zeros((SPC - NPC, FIN), np.float32)], axis=0)
        # tile-major: xp[p, t*FIN + f] = xv[t*128 + p, f]
        xp[:] = xv.reshape(TPC, 128, FIN).transpose(1, 0, 2).reshape(128, -1)
        bp = np.full((SPC,), NG, dtype=np.int32)
        bp[:NPC] = batch[nodes]
        bsb = bp.reshape(TPC, 128).T.astype(np.int32)  # [128, TPC]
        m = dict(common)
        m["xp"] = xp
        m["batchp"] = np.ascontiguousarray(bsb)
        m["idx16"] = np.ascontiguousarray(data["idx16"][c])
        m["intA"] = np.ascontiguousarray(data["intA"][c])
        m["intB"] = np.ascontiguousarray(data["intB"][c])
        in_maps.append(m)
    return in_maps


def build(cfg, sched, fin=3, collectives=True):
    C, TPC, SPC, TBL, T16 = cfg.C, cfg.TPC, cfg.SPC, cfg.TBL, sched["T16"]
    SA, SB = sched["SA"], sched["SB"]
    DA, DB = sched["DA"], sched["DB"]
    offA, offB = sched["offA"], sched["offB"]
    calls = sched["calls"]
    AL = mybir.AluOpType

    nc = bacc.Bacc(None, target_bir_lowering=False, debug=False, num_devices=C)

    # ---- kernel I/O ----
    xp_d = nc.dram_tensor("xp", [128, TPC * fin], F32, kind="ExternalInput")
    batch_d = nc.dram_tensor("batchp", [128, TPC], I32, kind="ExternalInput")
    idx16_d = nc.dram_tensor("idx16", [128, T16], I16, kind="ExternalInput")
    intA_d = nc.dram_tensor("intA", [128, SA], I32, kind="ExternalInput")
    intB_d = nc.dram_tensor("intB", [128, SB], I32, kind="ExternalInput")
    lp = []
    cins = [fin, 64, 128]
    for i in range(3):
        lp.append({
            "W": nc.dram_tensor(f"W{i}", [cins[i], 64], F32, kind="ExternalInput"),
            "b": nc.dram_tensor(f"b{i}", [64, 1], F32, kind="ExternalInput"),
            "g": nc.dram_tensor(f"g{i}", [64, 1], F32, kind="ExternalInput"),
            "beta": nc.dram_tensor(f"beta{i}", [64, 1], F32, kind="ExternalInput"),
            "rm": nc.dram_tensor(f"rm{i}", [64, 1], F32, kind="ExternalInput"),
            "rv": nc.dram_tensor(f"rv{i}", [64, 1], F32, kind="ExternalInput"),
        })
    encw0_d = nc.dram_tensor("enc_w0", [256, 128], F32, kind="ExternalInput")
    encb0_d = nc.dram_tensor("enc_b0", [128, 1], F32, kind="ExternalInput")
    encw1_d = nc.dram_tensor("enc_w1", [128, 64], F32, kind="ExternalInput")
    encb1_d = nc.dram_tensor("enc_b1", [64, 1], F32, kind="ExternalInput")
    decw0_d = nc.dram_tensor("dec_w0", [64, 32], F32, kind="ExternalInput")
    decb0_d = nc.dram_tensor("dec_b0", [32, 1], F32, kind="ExternalInput")
    decw1_d = nc.dram_tensor("dec_w1", [32, 1], F32, kind="ExternalInput")
    decb1_d = nc.dram_tensor("dec_b1", [1, 1], F32, kind="ExternalInput")
    out_d = nc.dram_tensor("out", [NG, 1], F32, kind="ExternalOutput")

    # ---- internal DRAM ----
    slab_d = nc.dram_tensor("slab", [SPC, 64], mybir.dt.bfloat16)
    table_bf_d = nc.dram_tensor("table_bf", [TBL, 64], mybir.dt.bfloat16, addr_space="Shared")
    table_d = nc.dram_tensor("table", [TBL, 64], F32)
    pool_in_d = nc.dram_tensor("pool_in", [NG, 65], F32)
    pool_out_d = nc.dram_tensor("pool_out", [NG, 65], F32, addr_space="Shared")
    pool_ag_d = nc.dram_tensor("pool_ag", [C * NG, 65], F32, addr_space="Shared")

    from contextlib import ExitStack
    with tile.TileContext(nc) as tc, ExitStack() as es:
        const = es.enter_context(tc.tile_pool(name="const", bufs=1))
        work = es.enter_context(tc.tile_pool(name="work", bufs=3))
        gpool = es.enter_context(tc.tile_pool(name="gath", bufs=2))
        pp = es.enter_context(tc.tile_pool(name="ps", bufs=3, space="PSUM"))
        ppacc = es.enter_context(tc.tile_pool(name="psacc", bufs=1, space="PSUM"))

        ident = const.tile([128, 128], F32, tag="ident")
        make_identity(nc, ident[:])
        ones_row = const.tile([1, 128], F32, tag="ones_row")
        nc.vector.memset(ones_row[:], 1.0)
        ones_col = const.tile([128, 1], F32, tag="ones_col")
        nc.vector.memset(ones_col[:], 1.0)
        iota8_i = const.tile([128, NG], I32, tag="iota8i")
        nc.gpsimd.iota(iota8_i[:], pattern=[[1, NG]], base=0, channel_multiplier=0)
        iota8 = const.tile([128, NG], F32, tag="iota8")
        nc.vector.tensor_copy(iota8[:], iota8_i[:])

        hfull = const.tile([128, TPC * 256], F32, tag="hfull")
        nc.gpsimd.memset(hfull[:], 0.0)
        msgsb = const.tile([128, TPC * 64], F32, tag="msgsb")
        nc.gpsimd.memset(msgsb[:], 0.0)
        slabsb = const.tile([128, TPC * 64], mybir.dt.bfloat16, tag="slabsb")
        dinv = const.tile([128, TPC], F32, tag="dinv")
        xsb = const.tile([128, TPC * fin], F32, tag="xsb")
        nc.sync.dma_start(out=xsb[:], in_=xp_d[:])
        batchsb = const.tile([128, TPC], I32, tag="batchsb")
        nc.sync.dma_start(out=batchsb[:], in_=batch_d[:])
        idx16sb = const.tile([128, T16], I16, tag="idx16sb")
        nc.sync.dma_start(out=idx16sb[:], in_=idx16_d[:])

        # ---- degree / dinv from pad counts ----
        _skip = set()
        intAsb = const.tile([128, SA], I32, tag="intAsb")
        nc.sync.dma_start(out=intAsb[:], in_=intA_d[:])
        intBsb = const.tile([128, SB], I32, tag="intBsb")
        nc.sync.dma_start(out=intBsb[:], in_=intB_d[:])
        if "deg" in _skip:
            nc.vector.memset(dinv[:], 1.0)
        for t in range(TPC if "deg" not in _skip else 0):
            da, db = int(DA[t]), int(DB[t])
            fa = work.tile([128, max(SA and da, db, 1)], F32, tag="degf")
            pa = work.tile([128, 1], F32, tag="pada")
            pb = work.tile([128, 1], F32, tag="padb")
            nc.vector.tensor_copy(fa[:, :da], intAsb[:, offA[t]:offA[t] + da])
            nc.vector.tensor_scalar(fa[:, :da], fa[:, :da], float(cfg.PAD_A),
                                    None, AL.is_equal)
            nc.vector.tensor_reduce(pa[:], fa[:, :da], axis=mybir.AxisListType.X,
                                    op=AL.add)
            nc.vector.tensor_copy(fa[:, :db], intBsb[:, offB[t]:offB[t] + db])
            nc.vector.tensor_scalar(fa[:, :db], fa[:, :db], float(cfg.PAD_B),
                                    None, AL.is_equal)
            nc.vector.tensor_reduce(pb[:], fa[:, :db], axis=mybir.AxisListType.X,
                                    op=AL.add)
            dg = work.tile([128, 1], F32, tag="dg")
            nc.vector.tensor_add(dg[:], pa[:], pb[:])
            # deg = (da+db) - pads, clamped to >= 1
            nc.vector.tensor_scalar(dg[:], dg[:], -1.0, float(da + db),
                                    AL.mult, AL.add)
            nc.vector.tensor_scalar_max(dg[:], dg[:], 1.0)
            nc.scalar.sqrt(dg[:], dg[:])
            nc.vector.reciprocal(dinv[:, t:t + 1], dg[:])
        if cfg.NPC % 128 != 0:
            # zero dinv on the dummy partitions of the last tile so dummy
            # slab rows (pad-gather targets) are always zero
            lastp = cfg.NPC % 128
            pidx_i = work.tile([128, 1], I32, tag="pidxi")
            nc.gpsimd.iota(pidx_i[:], pattern=[[1, 1]], base=0, channel_multiplier=1)
            pmask = work.tile([128, 1], F32, tag="pmask")
            nc.vector.tensor_copy(pmask[:], pidx_i[:])
            nc.vector.tensor_scalar(pmask[:], pmask[:], float(lastp), None, AL.is_lt)
            nc.vector.tensor_mul(dinv[:, TPC - 1:TPC], dinv[:, TPC - 1:TPC], pmask[:])

        # ---- fold BN into W' and per-feature bias; broadcast rows ----
        wps = []
        bbcs = []
        if "bn" in _skip:
            for i in range(3):
                cin = cins[i]
                wp = const.tile([cin, 64], F32, tag=f"wp_{i}")
                nc.sync.dma_start(out=wp[:], in_=lp[i]["W"][:])
                wps.append(wp)
                bbc = const.tile([128, 64], F32, tag=f"bbc_{i}")
                nc.vector.memset(bbc[:], 0.0)
                bbcs.append(bbc)
        for i in range(3 if "bn" not in _skip else 0):
            cin = cins[i]
            g64 = work.tile([64, 1], F32, tag="p64")
            rv64 = work.tile([64, 1], F32, tag="p64b")
            s64 = const.tile([64, 1], F32, tag=f"s64_{i}")
            nc.sync.dma_start(out=rv64[:], in_=lp[i]["rv"][:])
            nc.vector.tensor_scalar_add(rv64[:], rv64[:], BN_EPS)
            nc.scalar.sqrt(rv64[:], rv64[:])
            nc.vector.reciprocal(rv64[:], rv64[:])
            nc.sync.dma_start(out=g64[:], in_=lp[i]["g"][:])
            nc.vector.tensor_mul(s64[:], g64[:], rv64[:])
            # bias'' = (b - rm) * s + beta
            b64 = work.tile([64, 1], F32, tag="p64")
            rm64 = work.tile([64, 1], F32, tag="p64b")
            bb64 = const.tile([64, 1], F32, tag=f"bb64_{i}")
            nc.sync.dma_start(out=b64[:], in_=lp[i]["b"][:])
            nc.sync.dma_start(out=rm64[:], in_=lp[i]["rm"][:])
            nc.vector.tensor_sub(bb64[:], b64[:], rm64[:])
            nc.vector.tensor_mul(bb64[:], bb64[:], s64[:])
            be64 = work.tile([64, 1], F32, tag="p64")
            nc.sync.dma_start(out=be64[:], in_=lp[i]["beta"][:])
            nc.vector.tensor_add(bb64[:], bb64[:], be64[:])
            # transpose [64,1] -> [1,64], broadcast to [128,64]
            srow_ps = pp.tile([1, 64], F32, tag="ps", space="PSUM")
            nc.tensor.transpose(out=srow_ps[:], in_=s64[:], identity=ident[:64, :64])
            srow = work.tile([1, 64], F32, tag="row64")
            nc.vector.tensor_copy(srow[:], srow_ps[:])
            sbc_ps = pp.tile([128, 64], F32, tag="ps", space="PSUM")
            nc.tensor.matmul(out=sbc_ps[:], lhsT=ones_row[:1, :], rhs=srow[:],
                             start=True, stop=True)
            sbc = work.tile([128, 64], F32, tag="sbc")
            nc.vector.tensor_copy(sbc[:], sbc_ps[:])
            brow_ps = pp.tile([1, 64], F32, tag="ps", space="PSUM")
            nc.tensor.transpose(out=brow_ps[:], in_=bb64[:], identity=ident[:64, :64])
            brow = work.tile([1, 64], F32, tag="row64")
            nc.vector.tensor_copy(brow[:], brow_ps[:])
            bbc_ps = pp.tile([128, 64], F32, tag="ps", space="PSUM")
            nc.tensor.matmul(out=bbc_ps[:], lhsT=ones_row[:1, :], rhs=brow[:],
                             start=True, stop=True)
            bbc = const.tile([128, 64], F32, tag=f"bbc_{i}")
            nc.vector.tensor_copy(bbc[:], bbc_ps[:])
            bbcs.append(bbc)
            # W' = W * s (per output feature)
            wraw = work.tile([cin, 64], F32, tag="wraw")
            nc.sync.dma_start(out=wraw[:], in_=lp[i]["W"][:])
            wp = const.tile([cin, 64], F32, tag=f"wp_{i}")
            nc.vector.tensor_mul(wp[:], wraw[:], sbc[:cin, :])
            wps.append(wp)

        # encoder / decoder weights + broadcast biases
        encw0_a = const.tile([128, 128], F32, tag="encw0_a")
        encw0_b = const.tile([128, 128], F32, tag="encw0_b")
        encw0 = [encw0_a, encw0_b]
        nc.sync.dma_start(out=encw0[0][:], in_=encw0_d[0:128, :])
        nc.sync.dma_start(out=encw0[1][:], in_=encw0_d[128:256, :])
        encw1 = const.tile([128, 64], F32, tag="encw1")
        nc.sync.dma_start(out=encw1[:], in_=encw1_d[:])
        decw0 = const.tile([64, 32], F32, tag="decw0")
        nc.sync.dma_start(out=decw0[:], in_=decw0_d[:])
        decw1 = const.tile([32, 1], F32, tag="decw1")
        nc.sync.dma_start(out=decw1[:], in_=decw1_d[:])

        def bcast_bias(d_param, flen, parts, tag):
            v = work.tile([flen, 1], F32, tag="pbias")
            nc.sync.dma_start(out=v[:], in_=d_param[:])
            r_ps = pp.tile([1, flen], F32, tag="ps", space="PSUM")
            nc.tensor.transpose(out=r_ps[:], in_=v[:], identity=ident[:flen, :flen])
            r = work.tile([1, flen], F32, tag="rowb")
            nc.vector.tensor_copy(r[:], r_ps[:])
            b_ps = pp.tile([parts, flen], F32, tag="ps", space="PSUM")
            nc.tensor.matmul(out=b_ps[:], lhsT=ones_row[:1, :parts], rhs=r[:],
                             start=True, stop=True)
            b = const.tile([parts, flen], F32, tag=tag)
            nc.vector.tensor_copy(b[:], b_ps[:])
            return b

        encb0 = bcast_bias(encb0_d, 128, 128, "encb0")
        encb1 = bcast_bias(encb1_d, 64, 128, "encb1")
        decb0 = bcast_bias(decb0_d, 32, NG, "decb0")
        decb1 = bcast_bias(decb1_d, 1, NG, "decb1")

        # ---- GCN layers ----
        in_off = {0: None, 1: 0, 2: 64}
        wr_off = {0: (0, 64), 1: (128,), 2: (192,)}
        nlayers = 3
        if True:
         for li in range(nlayers):
            cin = cins[li]
            # (a) local hw slab = (h @ W') * dinv
            _mp = 4
            for t in range(TPC):
                if li == 0:
                    hin = xsb[:, t * fin:(t + 1) * fin]
                else:
                    o = t * 256 + in_off[li]
                    hin = hfull[:, o:o + cin]
                if _mp >= 1:
                    tp_ps = pp.tile([cin, 128], F32, tag="ps", space="PSUM")
                    nc.tensor.transpose(out=tp_ps[:], in_=hin, identity=ident[:])
                if _mp >= 2:
                    hT = work.tile([cin, 128], F32, tag="hT")
                    nc.vector.tensor_copy(hT[:], tp_ps[:])
                if _mp >= 3:
                    mm_ps = pp.tile([128, 64], F32, tag="ps", space="PSUM")
                    nc.tensor.matmul(out=mm_ps[:], lhsT=hT[:], rhs=wps[li][:],
                                     start=True, stop=True)
                if _mp >= 4:
                    nc.vector.tensor_scalar_mul(
                        slabsb[:, t * 64:(t + 1) * 64], mm_ps[:], dinv[:, t:t + 1])
                else:
                    nc.vector.memset(slabsb[:, t * 64:(t + 1) * 64], 0.01)
            # slab -> DRAM (row r = t*128+p  <->  sbuf [p, t*64+f])
            nc.sync.dma_start(
                out=slab_d[:].rearrange("(t p) f -> p t f", p=128),
                in_=slabsb[:].rearrange("p (t f) -> p t f", f=64))
            # (b) all-gather slabs into the table
            if collectives:
                nc.gpsimd.collective_compute(
                    "AllGather", AL.bypass,
                    replica_groups=[list(range(C))],
                    ins=[slab_d[:]],
                    outs=[table_bf_d[:]],
                )
            else:
                nc.sync.dma_start(out=table_bf_d[0:SPC, :].rearrange("(t p) f -> p t f", p=128),
                                  in_=slabsb[:].rearrange("p (t f) -> p t f", f=64))
            # upconvert to the f32 gather table (SWDGE cast DMA)
            nc.gpsimd.dma_start(out=table_d[:], in_=table_bf_d[:])
            # (c) gather + segment reduce + epilogue
            _sub = ""
            for call in calls:
                chunks, nidx = call["chunks"], call["nidx"]
                buf = gpool.tile([128, chunks, 64], F32, tag="gbuf")
                if call["win"] == 0:
                    in_view = table_d[0:cfg.WINA_ROWS, :]
                else:
                    in_view = table_d[cfg.WINB:TBL, :]
                c0 = call["idx_off"] // 16
                nc.gpsimd.dma_gather(
                    out_ap=buf[:],
                    in_ap=in_view,
                    idxs_ap=idx16sb[:, c0:c0 + nidx // 16],
                    num_idxs=nidx,
                    num_idxs_reg=nidx,
                    elem_size=64,
                    queue_num=0,
                    single_packet=False,
                )
                ci = 0
                for k, tt in enumerate(range(call["t0"], call["t0"] + call["ntiles"])):
                    d = call["tile_D"][k]
                    seg = buf[:, ci:ci + d, :].rearrange("p d f -> p f d")
                    ci += d
                    mslice = msgsb[:, tt * 64:(tt + 1) * 64]
                    if call["win"] == 0:
                        nc.vector.tensor_reduce(
                            mslice, seg, axis=mybir.AxisListType.X, op=AL.add)
                    else:
                        red = work.tile([128, 64], F32, tag="redB")
                        nc.vector.tensor_reduce(
                            red[:], seg, axis=mybir.AxisListType.X, op=AL.add)
                        nc.vector.tensor_add(mslice, mslice, red[:])
                        # epilogue for tile tt
                        e1 = work.tile([128, 64], F32, tag="epi")
                        nc.vector.tensor_scalar_mul(e1[:], mslice, dinv[:, tt:tt + 1])
                        nc.vector.tensor_add(e1[:], e1[:], bbcs[li][:])
                        w0 = wr_off[li][0]
                        nc.scalar.activation(
                            hfull[:, tt * 256 + w0:tt * 256 + w0 + 64], e1[:],
                            mybir.ActivationFunctionType.Relu)
                        if li == 0:
                            nc.vector.tensor_copy(
                                hfull[:, tt * 256 + 64:tt * 256 + 128],
                                hfull[:, tt * 256:tt * 256 + 64])

        # ---- encoder + pooling ----
        run_tail = True
        skip_ar = not collectives
        if run_tail:
            pool_ps = ppacc.tile([NG, 65], F32, tag="pool", space="PSUM")
        for t in range(TPC if run_tail else 0):
            h2 = hfull[:, t * 256:(t + 1) * 256]
            e1_ps = pp.tile([128, 128], F32, tag="ps", space="PSUM")
            for k in range(2):
                tp_ps = pp.tile([128, 128], F32, tag="ps2", space="PSUM")
                nc.tensor.transpose(out=tp_ps[:], in_=h2[:, k * 128:(k + 1) * 128],
                                    identity=ident[:])
                hT = work.tile([128, 128], F32, tag="hT2")
                nc.vector.tensor_copy(hT[:], tp_ps[:])
                nc.tensor.matmul(out=e1_ps[:], lhsT=hT[:], rhs=encw0[k][:],
                                 start=(k == 0), stop=(k == 1),
                                 skip_group_check=True)
            e1 = work.tile([128, 128], F32, tag="e1")
            nc.vector.tensor_add(e1[:], e1_ps[:], encb0[:])
            nc.scalar.activation(e1[:], e1[:], mybir.ActivationFunctionType.Relu)
            tp2_ps = pp.tile([128, 128], F32, tag="ps2", space="PSUM")
            nc.tensor.transpose(out=tp2_ps[:], in_=e1[:], identity=ident[:])
            e1T = work.tile([128, 128], F32, tag="hT2")
            nc.vector.tensor_copy(e1T[:], tp2_ps[:])
            e2_ps = pp.tile([128, 64], F32, tag="ps", space="PSUM")
            nc.tensor.matmul(out=e2_ps[:], lhsT=e1T[:], rhs=encw1[:],
                             start=True, stop=True, skip_group_check=True)
            e2 = work.tile([128, 65], F32, tag="e2")
            nc.vector.tensor_add(e2[:, :64], e2_ps[:], encb1[:])
            nc.scalar.activation(e2[:, :64], e2[:, :64],
                                 mybir.ActivationFunctionType.Relu)
            nc.vector.tensor_copy(e2[:, 64:65], ones_col[:])
            # one-hot over graphs
            btf = work.tile([128, 1], F32, tag="btf")
            nc.vector.tensor_copy(btf[:], batchsb[:, t:t + 1])
            oh = work.tile([128, NG], F32, tag="oh")
            nc.vector.tensor_tensor(
                out=oh[:], in0=btf[:].to_broadcast([128, NG]), in1=iota8[:],
                op=AL.is_equal)
            nc.tensor.matmul(out=pool_ps[:, :65], lhsT=oh[:], rhs=e2[:],
                             start=(t == 0), stop=(t == TPC - 1),
                             skip_group_check=True)
        if run_tail:
            poolsb = work.tile([NG, 65], F32, tag="poolsb")
            nc.vector.tensor_copy(poolsb[:], pool_ps[:])
            # ---- cross-core pool reduction: AllGather + local sum ----
            poolg = work.tile([NG, 65], F32, tag="poolg")
            if skip_ar:
                nc.vector.tensor_copy(poolg[:], poolsb[:])
            else:
                nc.sync.dma_start(out=pool_in_d[:], in_=poolsb[:])
                nc.gpsimd.collective_compute(
                    "AllGather", AL.bypass,
                    replica_groups=[list(range(C))],
                    ins=[pool_in_d[:]],
                    outs=[pool_ag_d[:]],
                )
                pall = work.tile([NG, C, 65], F32, tag="pall")
                nc.sync.dma_start(
                    out=pall[:], in_=pool_ag_d[:].rearrange("(c g) j -> g c j", g=NG))
                nc.vector.tensor_copy(poolg[:], pall[:, 0, :])
                for cc in range(1, C):
                    nc.vector.tensor_add(poolg[:], poolg[:], pall[:, cc, :])
            # gfeat = pool / max(counts, 1)
            cnt = work.tile([NG, 1], F32, tag="cnt")
            nc.vector.tensor_scalar_max(cnt[:], poolg[:, 64:65], 1.0)
            nc.vector.reciprocal(cnt[:], cnt[:])
            gfeat = work.tile([NG, 64], F32, tag="gfeat")
            nc.vector.tensor_scalar_mul(gfeat[:], poolg[:, :64], cnt[:])
            # ---- decoder ----
            gfT_ps = pp.tile([64, NG], F32, tag="ps", space="PSUM")
            nc.tensor.transpose(out=gfT_ps[:], in_=gfeat[:], identity=ident[:NG, :NG])
            gfT = work.tile([64, NG], F32, tag="gfT")
            nc.vector.tensor_copy(gfT[:], gfT_ps[:])
            o1_ps = pp.tile([NG, 32], F32, tag="ps", space="PSUM")
            nc.tensor.matmul(out=o1_ps[:], lhsT=gfT[:], rhs=decw0[:],
                             start=True, stop=True, skip_group_check=True)
            o1 = work.tile([NG, 32], F32, tag="o1")
            nc.vector.tensor_add(o1[:], o1_ps[:], decb0[:])
            nc.scalar.activation(o1[:], o1[:], mybir.ActivationFunctionType.Relu)
            o1T_ps = pp.tile([32, NG], F32, tag="ps", space="PSUM")
            nc.tensor.transpose(out=o1T_ps[:], in_=o1[:], identity=ident[:NG, :NG])
            o1T = work.tile([32, NG], F32, tag="o1T")
            nc.vector.tensor_copy(o1T[:], o1T_ps[:])
            o2_ps = pp.tile([NG, 1], F32, tag="ps", space="PSUM")
            nc.tensor.matmul(out=o2_ps[:], lhsT=o1T[:], rhs=decw1[:],
                             start=True, stop=True, skip_group_check=True)
            o2 = work.tile([NG, 1], F32, tag="o2")
            nc.vector.tensor_add(o2[:], o2_ps[:], decb1[:])
            nc.sync.dma_start(out=out_d[:], in_=o2[:])
        else:
            oz = work.tile([NG, 1], F32, tag="oz")
            nc.vector.memset(oz[:], 0.0)
            nc.sync.dma_start(out=out_d[:], in_=oz[:])

    nc.compile()
    return nc


_COMPILED = {}


def kernel(**inputs):
    """Full-input entry point: shards across 8 NeuronCores internally."""
    from concourse.bass_utils import run_bass_kernel_spmd

    cfg = CFG_FULL
    edge_index = np.asarray(inputs["edge_index"])
    batch = np.asarray(inputs["batch"])
    sched, data = preprocess(cfg, edge_index, batch)
    key = "full"
    if key not in _COMPILED:
        _COMPILED[key] = build(cfg, sched)
    nc = _COMPILED[key]
    in_maps = per_core_inputs(cfg, sched, data, inputs)
    res = run_bass_kernel_spmd(nc, in_maps, list(range(cfg.C)), trace=False)
    out = np.asarray(res.results[0]["out"])[:, 0].astype(np.float32)
    return out



# revision 5
# speedup vs baseline: 2.8571x; 2.8571x over previous
"""Sharded DenseGNN Bass kernel for 8 TRN2 NeuronCores (overlapped fp8 collectives).

Design:
  - Nodes partitioned across 8 cores (6250/core), degree-sorted into 49
    tiles of 128 (tile 31 holds only 106 real nodes + 22 dummy slots).
  - Tiles are split into 3 groups (17/15/17 tiles) whose table regions
    align exactly with the two int16 gather windows:
      g0 rows [0,17408)  g1 rows [17408,32768)  g2 rows [32768,50176)
      window A = [0,32768) = g0+g1   window B = [17409,50176) = g1+g2
  - Per GCN layer: per group, compute hw = (h @ W') * dinv, DMA the slab
    chunk out, AllGather it (fp8 e4m3) into tbf_g, then cast-DMA into the f32
    window tensors tA/tB (g1 goes into both).  Window-A gathers depend
    only on {AG0, AG1}; window-B gathers only on {AG1, AG2}.  The next
    layer's matmuls and AllGathers are emitted inside the current B-phase
    (one gather-call lag) so collectives overlap gather DMA; the encoder +
    pooling are emitted inside layer 2's B-phase for the same reason.
    tA/tB are double-buffered across layers.
  - Self-loop contribution is NOT gathered: msgs are initialised with the
    local dinv*hw value (Act engine) during the matmul phase.
  - Per-graph mean pooling via one-hot matmul, cross-core AllGather of the
    tiny [8, 65] pool partials, decoder computed redundantly on every core.

Structural preprocessing (index manipulation only) happens on the host;
the device computes dinv = rsqrt(deg) from host-counted integer degrees.
"""
import sys
import types

sys.path.insert(0, "/opt/trn_rl_repo")
if "antenv.axon_hooks" not in sys.modules:
    try:
        import antenv  # noqa: F401
        _m = types.ModuleType("antenv.axon_hooks")
        _m.get_axon_ntff_profile_hook = lambda: None
        sys.modules["antenv.axon_hooks"] = _m
    except ImportError:
        pass

import numpy as np

from concourse import bacc, bass, mybir, tile
from concourse.masks import make_identity

F32 = mybir.dt.float32
BF16 = mybir.dt.bfloat16
TDT = mybir.dt.float8e4  # collective transport dtype
I32 = mybir.dt.int32
I16 = mybir.dt.int16
NG = 8
BN_EPS = 1e-5


class Config:
    def __init__(self, n_nodes=50000, n_cores=8, max_idxs_per_call=12288):
        self.N = n_nodes
        self.C = n_cores
        self.NPC = n_nodes // n_cores          # 6250
        self.TPC = 49
        self.SPC = self.TPC * 128              # 6272
        self.TBL = self.C * self.SPC           # 50176
        # tile groups aligned with the two int16 windows
        self.GT = [(0, 17), (17, 32), (32, 49)]
        self.GS = [2176, 1920, 2176]           # per-core slots per group
        self.SOFF = [0, 2176, 4096]
        self.GOFF = [0, 17408, 32768, 50176]   # table row offsets
        self.DUMMY_TILE = 31
        self.DUMMY_P0 = 106                    # partitions >= this are dummy
        self.WINB = self.TBL - 32767           # 17409
        self.WA_ROWS = 32768
        self.WB_ROWS = 32767
        # pad rows point at the (always zero) core-0 dummy row in g1
        self.PAD_ROW = self.GOFF[1] + 1898     # 19306
        self.PAD_A = self.PAD_ROW              # < 32768
        self.PAD_B = self.PAD_ROW - self.WINB  # 11897
        assert self.PAD_A < self.WA_ROWS
        assert 0 <= self.PAD_B < self.WB_ROWS
        self.MAX_IDXS = max_idxs_per_call


CFG_FULL = Config()


def preprocess(cfg, edge_index, batch):
    """Build the SPMD-uniform schedule + per-core index data."""
    src_g = np.asarray(edge_index[0], dtype=np.int64)
    dst_g = np.asarray(edge_index[1], dtype=np.int64)
    N, C, SPC, TPC = cfg.N, cfg.C, cfg.SPC, cfg.TPC

    deg_in = np.bincount(dst_g, minlength=N).astype(np.int64)
    order_g = np.argsort(-deg_in, kind="stable")

    GS = np.array(cfg.GS)
    SOFF = np.array(cfg.SOFF)
    GOFF = np.array(cfg.GOFF[:3])
    slot_group = np.zeros(SPC, dtype=np.int64)
    for g in range(3):
        slot_group[cfg.SOFF[g]:cfg.SOFF[g] + cfg.GS[g]] = g

    perm = np.full((C, SPC), -1, dtype=np.int64)
    core_of = np.empty(N, dtype=np.int64)
    slot_of = np.empty(N, dtype=np.int64)
    pos = 0
    for k in range(TPC):
        bw = cfg.DUMMY_P0 if k == cfg.DUMMY_TILE else 128
        blk = order_g[pos:pos + C * bw]
        pos += C * bw
        for c in range(C):
            nodes = blk[c * bw:(c + 1) * bw]
            perm[c, k * 128:k * 128 + bw] = nodes
            core_of[nodes] = c
            slot_of[nodes] = k * 128 + np.arange(bw)
    assert pos == N

    sg = slot_group[slot_of]
    gpos = GOFF[sg] + core_of * GS[sg] + (slot_of - SOFF[sg])

    sgp = gpos[src_g]
    dst_core = core_of[dst_g]
    dst_slot = slot_of[dst_g]

    # classify for window split: 0=mustA, 1=flex, 2=mustB
    cls = np.ones(src_g.shape, dtype=np.int8)
    cls[sgp < cfg.WINB] = 0
    cls[sgp > 32767] = 2

    per_core = []
    dA_all = np.zeros((C, SPC), dtype=np.int64)
    dB_all = np.zeros((C, SPC), dtype=np.int64)
    for c in range(C):
        m = dst_core == c
        e_sgp = sgp[m]
        e_cls = cls[m]
        e_slot = dst_slot[m]
        order = np.lexsort((e_cls, e_slot))
        e_sgp = e_sgp[order]
        e_cls = e_cls[order]
        e_slot = e_slot[order]
        counts = np.bincount(e_slot, minlength=SPC)
        offs = np.concatenate([[0], np.cumsum(counts)])
        nA_must = np.bincount(e_slot[e_cls == 0], minlength=SPC)
        nflex = np.bincount(e_slot[e_cls == 1], minlength=SPC)
        need_a = np.clip((counts + 1) // 2 - nA_must, 0, nflex)
        dA = nA_must + need_a
        dB = counts - dA
        dA_all[c] = dA
        dB_all[c] = dB
        per_core.append(dict(e_sgp=e_sgp, offs=offs, dA=dA, dB=dB))

    # cross-core uniform per-tile slot widths
    DA = np.maximum(dA_all.reshape(C, TPC, 128).max(axis=(0, 2)), 1)
    DB = np.maximum(dB_all.reshape(C, TPC, 128).max(axis=(0, 2)), 1)

    # per-core padded slot tables [128, sum(D)] int32 (window-local indices)
    SA, SB = int(DA.sum()), int(DB.sum())
    offA = np.concatenate([[0], np.cumsum(DA)]).astype(np.int64)
    offB = np.concatenate([[0], np.cumsum(DB)]).astype(np.int64)
    intA = np.full((C, 128, SA), cfg.PAD_A, dtype=np.int32)
    intB = np.full((C, 128, SB), cfg.PAD_B, dtype=np.int32)
    for c in range(C):
        pc = per_core[c]
        e_sgp, offs, dA, dB = pc["e_sgp"], pc["offs"], pc["dA"], pc["dB"]
        node_of_edge = np.repeat(np.arange(SPC), np.diff(offs))
        rank = np.arange(e_sgp.size) - offs[node_of_edge]
        isA = rank < dA[node_of_edge]
        t_of_node = node_of_edge // 128
        p_of_node = node_of_edge % 128
        ra = rank[isA]
        intA[c, p_of_node[isA], offA[t_of_node[isA]] + ra] = e_sgp[isA]
        rb = rank[~isA] - dA[node_of_edge[~isA]]
        intB[c, p_of_node[~isA], offB[t_of_node[~isA]] + rb] = (
            e_sgp[~isA] - cfg.WINB)

    # gather-call schedule (uniform): batch consecutive tiles per window
    calls = []
    idx_off = 0
    for win, D, off in ((0, DA, offA), (1, DB, offB)):
        t = 0
        while t < TPC:
            t0, nidx, chunks = t, 0, 0
            while t < TPC:
                d = int(D[t])
                if nidx + d * 128 > cfg.MAX_IDXS and t > t0:
                    break
                chunks += d
                nidx += d * 128
                t += 1
            calls.append(dict(win=win, t0=t0, ntiles=t - t0, idx_off=idx_off,
                              nidx=nidx, chunks=chunks,
                              tile_D=[int(D[tt]) for tt in range(t0, t)]))
            idx_off += nidx
    total_idx = idx_off
    assert total_idx % 128 == 0
    T16 = total_idx // 16

    # per-core wrapped int16 idx lists
    idx16 = np.empty((C, 128, T16), dtype=np.int16)
    for c in range(C):
        flat = np.empty(total_idx, dtype=np.int16)
        for call in calls:
            win = call["win"]
            tbl = intA[c] if win == 0 else intB[c]
            off = offA if win == 0 else offB
            p = call["idx_off"]
            for k, tt in enumerate(range(call["t0"], call["t0"] + call["ntiles"])):
                d = call["tile_D"][k]
                blk = tbl[:, off[tt]:off[tt] + d]      # [128, d]
                flat[p:p + d * 128] = blk.T.reshape(-1).astype(np.int16)
                p += d * 128
            assert p == call["idx_off"] + call["nidx"]
        wrapped = flat.reshape(-1, 16).T               # [16, T16]
        idx16[c] = np.tile(wrapped, (8, 1))

    # per-core degree (incl. self loop) in tile-major layout, 1 on dummies
    degp = np.empty((C, 128, TPC), dtype=np.int32)
    for c in range(C):
        nodes = perm[c]
        dv = np.ones(SPC, dtype=np.int64)
        valid = nodes >= 0
        dv[valid] = deg_in[nodes[valid]] + 1
        degp[c] = dv.reshape(TPC, 128).T

    sched = dict(cfg=cfg, DA=DA, DB=DB, SA=SA, SB=SB,
                 offA=offA, offB=offB, calls=calls, T16=T16,
                 perm=perm, gpos=gpos)
    data = dict(idx16=idx16, degp=degp)
    return sched, data


def per_core_inputs(cfg, sched, data, inputs):
    """Build in_maps for run_bass_kernel_spmd."""
    x = np.asarray(inputs["x"], dtype=np.float32)
    batch = np.asarray(inputs["batch"], dtype=np.int64)
    perm = sched["perm"]
    C, SPC, TPC = cfg.C, cfg.SPC, cfg.TPC
    FIN = x.shape[1]

    common = {}
    for i in range(3):
        common[f"W{i}"] = np.ascontiguousarray(inputs[f"W{i}"], dtype=np.float32)
        for k in ("b", "g", "beta", "rm", "rv"):
            common[f"{k}{i}"] = np.ascontiguousarray(
                np.asarray(inputs[f"{k}{i}"], dtype=np.float32).reshape(-1, 1))
    common["enc_w0"] = np.ascontiguousarray(inputs["enc_w0"], dtype=np.float32)
    common["enc_w1"] = np.ascontiguousarray(inputs["enc_w1"], dtype=np.float32)
    common["dec_w0"] = np.ascontiguousarray(inputs["dec_w0"], dtype=np.float32)
    common["dec_w1"] = np.ascontiguousarray(inputs["dec_w1"], dtype=np.float32)
    for k in ("enc_b0", "enc_b1", "dec_b0", "dec_b1"):
        common[k] = np.ascontiguousarray(
            np.asarray(inputs[k], dtype=np.float32).reshape(-1, 1))

    in_maps = []
    for c in range(C):
        nodes = perm[c]                                # [SPC] with -1 dummies
        valid = nodes >= 0
        xv = np.zeros((SPC, FIN), dtype=np.float32)
        xv[valid] = x[nodes[valid]]
        xp = xv.reshape(TPC, 128, FIN).transpose(1, 0, 2).reshape(128, -1)
        bp = np.full((SPC,), NG, dtype=np.int64)
        bp[valid] = batch[nodes[valid]]
        bsb = bp.reshape(TPC, 128).T.astype(np.int32)  # [128, TPC]
        m = dict(common)
        m["xp"] = np.ascontiguousarray(xp)
        m["batchp"] = np.ascontiguousarray(bsb)
        m["idx16"] = np.ascontiguousarray(data["idx16"][c])
        m["degp"] = np.ascontiguousarray(data["degp"][c])
        in_maps.append(m)
    return in_maps


def build(cfg, sched, fin=3, collectives=True):
    C, TPC, SPC, TBL, T16 = cfg.C, cfg.TPC, cfg.SPC, cfg.TBL, sched["T16"]
    calls = sched["calls"]
    AL = mybir.AluOpType
    AF = mybir.ActivationFunctionType

    nc = bacc.Bacc(None, target_bir_lowering=False, debug=False, num_devices=C)

    # ---- kernel I/O ----
    xp_d = nc.dram_tensor("xp", [128, TPC * fin], F32, kind="ExternalInput")
    batch_d = nc.dram_tensor("batchp", [128, TPC], I32, kind="ExternalInput")
    idx16_d = nc.dram_tensor("idx16", [128, T16], I16, kind="ExternalInput")
    degp_d = nc.dram_tensor("degp", [128, TPC], I32, kind="ExternalInput")
    lp = []
    cins = [fin, 64, 128]
    for i in range(3):
        lp.append({
            "W": nc.dram_tensor(f"W{i}", [cins[i], 64], F32, kind="ExternalInput"),
            "b": nc.dram_tensor(f"b{i}", [64, 1], F32, kind="ExternalInput"),
            "g": nc.dram_tensor(f"g{i}", [64, 1], F32, kind="ExternalInput"),
            "beta": nc.dram_tensor(f"beta{i}", [64, 1], F32, kind="ExternalInput"),
            "rm": nc.dram_tensor(f"rm{i}", [64, 1], F32, kind="ExternalInput"),
            "rv": nc.dram_tensor(f"rv{i}", [64, 1], F32, kind="ExternalInput"),
        })
    encw0_d = nc.dram_tensor("enc_w0", [256, 128], F32, kind="ExternalInput")
    encb0_d = nc.dram_tensor("enc_b0", [128, 1], F32, kind="ExternalInput")
    encw1_d = nc.dram_tensor("enc_w1", [128, 64], F32, kind="ExternalInput")
    encb1_d = nc.dram_tensor("enc_b1", [64, 1], F32, kind="ExternalInput")
    decw0_d = nc.dram_tensor("dec_w0", [64, 32], F32, kind="ExternalInput")
    decb0_d = nc.dram_tensor("dec_b0", [32, 1], F32, kind="ExternalInput")
    decw1_d = nc.dram_tensor("dec_w1", [32, 1], F32, kind="ExternalInput")
    decb1_d = nc.dram_tensor("dec_b1", [1, 1], F32, kind="ExternalInput")
    out_d = nc.dram_tensor("out", [NG, 1], F32, kind="ExternalOutput")

    # ---- internal DRAM ----
    GS = cfg.GS
    slab_gd = [nc.dram_tensor(f"slab{g}", [GS[g], 64], TDT) for g in range(3)]
    tbf_gd = [nc.dram_tensor(f"tbf{g}", [C * GS[g], 64], TDT,
                             addr_space="Shared") for g in range(3)]
    tA_d = [nc.dram_tensor(f"tA{p}", [cfg.WA_ROWS, 64], F32) for p in range(2)]
    tB_d = [nc.dram_tensor(f"tB{p}", [cfg.WB_ROWS, 64], F32) for p in range(2)]
    pool_in_d = nc.dram_tensor("pool_in", [NG, 65], F32)
    pool_ag_d = nc.dram_tensor("pool_ag", [C * NG, 65], F32, addr_space="Shared")

    G1N = cfg.GOFF[2] - cfg.GOFF[1]            # 15360 rows in g1

    from contextlib import ExitStack
    with tile.TileContext(nc) as tc, ExitStack() as es:
        const = es.enter_context(tc.tile_pool(name="const", bufs=1))
        work = es.enter_context(tc.tile_pool(name="work", bufs=3))
        gpool = es.enter_context(tc.tile_pool(name="gath", bufs=2))
        pp = es.enter_context(tc.tile_pool(name="ps", bufs=3, space="PSUM"))
        pp2 = es.enter_context(tc.tile_pool(name="ps2", bufs=4, space="PSUM"))
        ppacc = es.enter_context(tc.tile_pool(name="psacc", bufs=1, space="PSUM"))

        ident = const.tile([128, 128], F32, tag="ident")
        make_identity(nc, ident[:])
        ones_row = const.tile([1, 128], F32, tag="ones_row")
        nc.vector.memset(ones_row[:], 1.0)
        ones_col = const.tile([128, 1], F32, tag="ones_col")
        nc.vector.memset(ones_col[:], 1.0)
        iota8_i = const.tile([128, TPC * NG], I32, tag="iota8i")
        nc.gpsimd.iota(iota8_i[:], pattern=[[0, TPC], [1, NG]], base=0,
                       channel_multiplier=0)
        iota8 = const.tile([128, TPC * NG], F32, tag="iota8")
        nc.vector.tensor_copy(iota8[:], iota8_i[:])

        hfull = const.tile([128, TPC * 256], F32, tag="hfull")
        msgsb = const.tile([128, TPC * 64], F32, tag="msgsb")
        slabsb = const.tile([128, TPC * 64], TDT, tag="slabsb")
        dinv = const.tile([128, TPC], F32, tag="dinv")
        xsb = const.tile([128, TPC * fin], F32, tag="xsb")
        nc.sync.dma_start(out=xsb[:], in_=xp_d[:])
        batchsb = const.tile([128, TPC], I32, tag="batchsb")
        nc.sync.dma_start(out=batchsb[:], in_=batch_d[:])
        # one-hot graph membership for ALL tiles in one op: [128, TPC*NG]
        batchf = const.tile([128, TPC], F32, tag="batchf")
        nc.vector.tensor_copy(batchf[:], batchsb[:])
        oh_all = const.tile([128, TPC * NG], F32, tag="oh_all")
        nc.vector.tensor_tensor(
            out=oh_all[:].rearrange("p (t o) -> p t o", o=NG),
            in0=batchf[:].rearrange("p (t o) -> p t o", o=1).to_broadcast(
                [128, TPC, NG]),
            in1=iota8[:].rearrange("p (t o) -> p t o", o=NG),
            op=AL.is_equal)
        idx16sb = const.tile([128, T16], I16, tag="idx16sb")
        nc.sync.dma_start(out=idx16sb[:], in_=idx16_d[:])

        # ---- dinv = rsqrt(deg) from host-counted degrees ----
        degsb = work.tile([128, TPC], I32, tag="degsb")
        nc.sync.dma_start(out=degsb[:], in_=degp_d[:])
        nc.vector.tensor_copy(dinv[:], degsb[:])
        nc.scalar.sqrt(dinv[:], dinv[:])
        nc.vector.reciprocal(dinv[:], dinv[:])
        # zero dinv on the dummy partitions of the dummy tile so those slab
        # rows (pad-gather targets) are always zero
        dt_, dp0 = cfg.DUMMY_TILE, cfg.DUMMY_P0
        pidx_i = work.tile([128, 1], I32, tag="pidxi")
        nc.gpsimd.iota(pidx_i[:], pattern=[[1, 1]], base=0, channel_multiplier=1)
        pmask = work.tile([128, 1], F32, tag="pmask")
        nc.vector.tensor_copy(pmask[:], pidx_i[:])
        nc.vector.tensor_scalar(pmask[:], pmask[:], float(dp0), None, AL.is_lt)
        nc.vector.tensor_mul(dinv[:, dt_:dt_ + 1], dinv[:, dt_:dt_ + 1], pmask[:])

        # ---- BN fold: W' = W * s, bias'' = (b - rm) * s + beta ----
        wps = [None, None, None]
        bbcs = [None, None, None]

        def emit_bn_fold(i):
            cin = cins[i]
            g64 = work.tile([64, 1], F32, tag="p64")
            rv64 = work.tile([64, 1], F32, tag="p64b")
            s64 = const.tile([64, 1], F32, tag=f"s64_{i}")
            nc.sync.dma_start(out=rv64[:], in_=lp[i]["rv"][:])
            nc.vector.tensor_scalar_add(rv64[:], rv64[:], BN_EPS)
            nc.scalar.sqrt(rv64[:], rv64[:])
            nc.vector.reciprocal(rv64[:], rv64[:])
            nc.sync.dma_start(out=g64[:], in_=lp[i]["g"][:])
            nc.vector.tensor_mul(s64[:], g64[:], rv64[:])
            b64 = work.tile([64, 1], F32, tag="p64")
            rm64 = work.tile([64, 1], F32, tag="p64b")
            bb64 = const.tile([64, 1], F32, tag=f"bb64_{i}")
            nc.sync.dma_start(out=b64[:], in_=lp[i]["b"][:])
            nc.sync.dma_start(out=rm64[:], in_=lp[i]["rm"][:])
            nc.vector.tensor_sub(bb64[:], b64[:], rm64[:])
            nc.vector.tensor_mul(bb64[:], bb64[:], s64[:])
            be64 = work.tile([64, 1], F32, tag="p64")
            nc.sync.dma_start(out=be64[:], in_=lp[i]["beta"][:])
            nc.vector.tensor_add(bb64[:], bb64[:], be64[:])
            srow_ps = pp.tile([1, 64], F32, tag="ps", space="PSUM")
            nc.tensor.transpose(out=srow_ps[:], in_=s64[:], identity=ident[:64, :64])
            srow = work.tile([1, 64], F32, tag="row64")
            nc.vector.tensor_copy(srow[:], srow_ps[:])
            sbc_ps = pp.tile([128, 64], F32, tag="ps", space="PSUM")
            nc.tensor.matmul(out=sbc_ps[:], lhsT=ones_row[:1, :], rhs=srow[:],
                             start=True, stop=True)
            sbc = work.tile([128, 64], F32, tag="sbc")
            nc.vector.tensor_copy(sbc[:], sbc_ps[:])
            brow_ps = pp.tile([1, 64], F32, tag="ps", space="PSUM")
            nc.tensor.transpose(out=brow_ps[:], in_=bb64[:], identity=ident[:64, :64])
            brow = work.tile([1, 64], F32, tag="row64")
            nc.vector.tensor_copy(brow[:], brow_ps[:])
            bbc_ps = pp.tile([128, 64], F32, tag="ps", space="PSUM")
            nc.tensor.matmul(out=bbc_ps[:], lhsT=ones_row[:1, :], rhs=brow[:],
                             start=True, stop=True)
            bbc = const.tile([128, 64], F32, tag=f"bbc_{i}")
            nc.vector.tensor_copy(bbc[:], bbc_ps[:])
            bbcs[i] = bbc
            wraw = work.tile([cin, 64], F32, tag="wraw")
            nc.sync.dma_start(out=wraw[:], in_=lp[i]["W"][:])
            wp = const.tile([cin, 64], F32, tag=f"wp_{i}")
            nc.vector.tensor_mul(wp[:], wraw[:], sbc[:cin, :])
            wps[i] = wp

        in_off = {0: None, 1: 0, 2: 64}
        wr_off = {0: 0, 1: 128, 2: 192}

        def emit_mm_tile(li, t):
            cin = cins[li]
            if li == 0:
                hin = xsb[:, t * fin:(t + 1) * fin]
            else:
                o = t * 256 + in_off[li]
                hin = hfull[:, o:o + cin]
            tp_ps = pp.tile([cin, 128], F32, tag="ps", space="PSUM")
            nc.tensor.transpose(out=tp_ps[:], in_=hin, identity=ident[:])
            hT = work.tile([cin, 128], F32, tag="hT")
            nc.scalar.copy(hT[:], tp_ps[:])
            mm_ps = pp.tile([128, 64], F32, tag="ps", space="PSUM")
            nc.tensor.matmul(out=mm_ps[:], lhsT=hT[:], rhs=wps[li][:],
                             start=True, stop=True)
            dv = dinv[:, t:t + 1]
            nc.scalar.activation(slabsb[:, t * 64:(t + 1) * 64], mm_ps[:],
                                 AF.Copy, scale=dv)
            # msgs initialised with the self-loop term dinv*hw
            nc.scalar.activation(msgsb[:, t * 64:(t + 1) * 64], mm_ps[:],
                                 AF.Copy, scale=dv)

        def emit_post_group(li, g):
            t0g, t1g = cfg.GT[g]
            nc.sync.dma_start(
                out=slab_gd[g][:].rearrange("(t p) f -> p t f", p=128),
                in_=slabsb[:, t0g * 64:t1g * 64].rearrange(
                    "p (t f) -> p t f", f=64))
            if collectives:
                nc.gpsimd.collective_compute(
                    "AllGather", AL.bypass,
                    replica_groups=[list(range(C))],
                    ins=[slab_gd[g][:]],
                    outs=[tbf_gd[g][:]],
                )
            else:
                nc.sync.dma_start(out=tbf_gd[g][0:GS[g], :], in_=slab_gd[g][:])

        # encoder weights + broadcast biases (emitted in preamble)
        def emit_encdec_loads():
            nc.sync.dma_start(out=encw01[:], in_=encw0_d[0:128, :])
            nc.sync.dma_start(out=encw2[:], in_=encw0_d[128:192, :])
            nc.sync.dma_start(out=encw3[:], in_=encw0_d[192:256, :])
            nc.sync.dma_start(out=encw1[:], in_=encw1_d[:])
            nc.sync.dma_start(out=decw0[:], in_=decw0_d[:])
            nc.sync.dma_start(out=decw1[:], in_=decw1_d[:])

        encw01 = const.tile([128, 128], F32, tag="encw01")
        encw2 = const.tile([64, 128], F32, tag="encw2")
        encw3 = const.tile([64, 128], F32, tag="encw3")
        encw1 = const.tile([128, 64], F32, tag="encw1")
        decw0 = const.tile([64, 32], F32, tag="decw0")
        decw1 = const.tile([32, 1], F32, tag="decw1")
        e1part = const.tile([128, TPC * 128], F32, tag="e1part")

        def bcast_bias(d_param, flen, parts, tag):
            """Returns (broadcast [parts, flen], row [1, flen])."""
            v = work.tile([flen, 1], F32, tag="pbias")
            nc.sync.dma_start(out=v[:], in_=d_param[:])
            r_ps = pp.tile([1, flen], F32, tag="ps", space="PSUM")
            nc.tensor.transpose(out=r_ps[:], in_=v[:], identity=ident[:flen, :flen])
            r = const.tile([1, flen], F32, tag=f"{tag}_row")
            nc.vector.tensor_copy(r[:], r_ps[:])
            b_ps = pp.tile([parts, flen], F32, tag="ps", space="PSUM")
            nc.tensor.matmul(out=b_ps[:], lhsT=ones_row[:1, :parts], rhs=r[:],
                             start=True, stop=True)
            b = const.tile([parts, flen], F32, tag=tag)
            nc.vector.tensor_copy(b[:], b_ps[:])
            return b, r

        # ---- encoder + pooling, per tile (split across layer-2 A/B phases) ----
        pool_ps = ppacc.tile([NG, 65], F32, tag="pool", space="PSUM")

        def emit_enc_partial(t):
            # e1part = encb0 + [h1 h1]^T w[0:128] + h2^T w[128:192]
            # (h1/h2 are layer-0/1 outputs; available during layer 2's A phase)
            h2 = hfull[:, t * 256:(t + 1) * 256]
            tpa_ps = pp2.tile([128, 128], F32, tag="ps2", space="PSUM")
            nc.tensor.transpose(out=tpa_ps[:], in_=h2[:, 0:128], identity=ident[:])
            hTa = work.tile([128, 128], F32, tag="hT2")
            nc.scalar.copy(hTa[:], tpa_ps[:])
            tpb_ps = pp2.tile([64, 128], F32, tag="ps2", space="PSUM")
            nc.tensor.transpose(out=tpb_ps[:], in_=h2[:, 128:192], identity=ident[:])
            hTb = work.tile([64, 128], F32, tag="hTb")
            nc.scalar.copy(hTb[:], tpb_ps[:])
            e1p_ps = pp.tile([128, 128], F32, tag="ps", space="PSUM")
            nc.tensor.matmul(out=e1p_ps[:], lhsT=ones_row[:1, :], rhs=encb0_row[:],
                             start=True, stop=False, skip_group_check=True)
            nc.tensor.matmul(out=e1p_ps[:], lhsT=hTa[:], rhs=encw01[:],
                             start=False, stop=False, skip_group_check=True)
            nc.tensor.matmul(out=e1p_ps[:], lhsT=hTb[:], rhs=encw2[:],
                             start=False, stop=True, skip_group_check=True)
            nc.scalar.copy(e1part[:, t * 128:(t + 1) * 128], e1p_ps[:])

        def emit_encoder_tile(t):
            # finish e1 with the h3 contribution, then e2 + pooling
            h2 = hfull[:, t * 256:(t + 1) * 256]
            tpc_ps = pp2.tile([64, 128], F32, tag="ps2", space="PSUM")
            nc.tensor.transpose(out=tpc_ps[:], in_=h2[:, 192:256], identity=ident[:])
            hTc = work.tile([64, 128], F32, tag="hTb")
            nc.scalar.copy(hTc[:], tpc_ps[:])
            e1_ps = pp.tile([128, 128], F32, tag="ps", space="PSUM")
            nc.tensor.matmul(out=e1_ps[:], lhsT=ident[:],
                             rhs=e1part[:, t * 128:(t + 1) * 128],
                             start=True, stop=False, skip_group_check=True)
            nc.tensor.matmul(out=e1_ps[:], lhsT=hTc[:], rhs=encw3[:],
                             start=False, stop=True, skip_group_check=True)
            e1 = work.tile([128, 128], F32, tag="e1")
            nc.scalar.activation(e1[:], e1_ps[:], AF.Relu)
            tp2_ps = pp2.tile([128, 128], F32, tag="ps2", space="PSUM")
            nc.tensor.transpose(out=tp2_ps[:], in_=e1[:], identity=ident[:])
            e1T = work.tile([128, 128], F32, tag="hT2")
            nc.scalar.copy(e1T[:], tp2_ps[:])
            e2_ps = pp.tile([128, 64], F32, tag="ps", space="PSUM")
            nc.tensor.matmul(out=e2_ps[:], lhsT=ones_row[:1, :], rhs=encb1_row[:],
                             start=True, stop=False, skip_group_check=True)
            nc.tensor.matmul(out=e2_ps[:], lhsT=e1T[:], rhs=encw1[:],
                             start=False, stop=True, skip_group_check=True)
            e2 = work.tile([128, 65], F32, tag="e2")
            nc.scalar.activation(e2[:, :64], e2_ps[:], AF.Relu)
            nc.vector.tensor_copy(e2[:, 64:65], ones_col[:])
            nc.tensor.matmul(out=pool_ps[:, :65],
                             lhsT=oh_all[:, t * NG:(t + 1) * NG], rhs=e2[:],
                             start=(t == 0), stop=(t == TPC - 1),
                             skip_group_check=True)

        # =========================== emission ===========================
        emit_bn_fold(0)
        for t in range(*cfg.GT[0]):
            emit_mm_tile(0, t)
        emit_post_group(0, 0)
        emit_bn_fold(1)
        emit_bn_fold(2)
        for t in range(*cfg.GT[1]):
            emit_mm_tile(0, t)
        emit_post_group(0, 1)
        emit_encdec_loads()
        encb0, encb0_row = bcast_bias(encb0_d, 128, 128, "encb0")
        encb1, encb1_row = bcast_bias(encb1_d, 64, 128, "encb1")
        decb0, _ = bcast_bias(decb0_d, 32, NG, "decb0")
        decb1, _ = bcast_bias(decb1_d, 1, NG, "decb1")
        for t in range(*cfg.GT[2]):
            emit_mm_tile(0, t)
        emit_post_group(0, 2)

        callsA = [c for c in calls if c["win"] == 0]
        callsB = [c for c in calls if c["win"] == 1]
        group_end = {cfg.GT[g][1] - 1: g for g in range(3)}

        AOFF = 3  # A-calls emitted ahead of the interleaved B stream

        for li in range(3):
            tAc, tBc = tA_d[li % 2], tB_d[li % 2]
            # upconverts needed by the A window (g0 + g1)
            nc.gpsimd.dma_start(out=tAc[0:cfg.GOFF[1], :], in_=tbf_gd[0][:])
            nc.gpsimd.dma_start(out=tAc[cfg.GOFF[1]:cfg.GOFF[2], :],
                                in_=tbf_gd[1][:])
            pending = []

            def flush_pending():
                while pending:
                    emit_post_group(*pending.pop(0))

            a_covered = -1  # max tile whose A contribution has been emitted

            def emit_call(call, win):
                nonlocal a_covered
                chunks, nidx = call["chunks"], call["nidx"]
                buf = gpool.tile([128, chunks, 64], F32, tag="gbuf")
                in_view = tAc[:] if win == 0 else tBc[:]
                c0 = call["idx_off"] // 16
                nc.gpsimd.dma_gather(
                    out_ap=buf[:],
                    in_ap=in_view,
                    idxs_ap=idx16sb[:, c0:c0 + nidx // 16],
                    num_idxs=nidx,
                    num_idxs_reg=nidx,
                    elem_size=64,
                    queue_num=0,
                    single_packet=False,
                )
                flush_pending()
                ci = 0
                for k in range(call["ntiles"]):
                    tt = call["t0"] + k
                    d = call["tile_D"][k]
                    seg = buf[:, ci:ci + d, :].rearrange("p d f -> p f d")
                    ci += d
                    mslice = msgsb[:, tt * 64:(tt + 1) * 64]
                    red = work.tile([128, 64], F32, tag="red")
                    nc.vector.tensor_reduce(
                        red[:], seg, axis=mybir.AxisListType.X, op=AL.add)
                    nc.vector.tensor_add(mslice, mslice, red[:])
                    if win == 0:
                        a_covered = tt
                        if li == 2:
                            emit_enc_partial(tt)
                    else:
                        assert tt <= a_covered, (tt, a_covered)
                        # epilogue: h = relu(msgs*dinv + bias)
                        w0 = wr_off[li]
                        hslice = hfull[:, tt * 256 + w0:tt * 256 + w0 + 64]
                        nc.vector.scalar_tensor_tensor(
                            out=hslice, in0=mslice, scalar=dinv[:, tt:tt + 1],
                            in1=bbcs[li][:], op0=AL.mult, op1=AL.add)
                        nc.scalar.activation(hslice, hslice, AF.Relu)
                        if li == 0:
                            nc.scalar.copy(
                                hfull[:, tt * 256 + 64:tt * 256 + 128], hslice)
                        if li < 2:
                            emit_mm_tile(li + 1, tt)
                            if tt in group_end:
                                pending.append((li + 1, group_end[tt]))
                        else:
                            emit_encoder_tile(tt)

            for call in callsA[:AOFF]:
                emit_call(call, 0)
            # upconverts needed by the B window (g1 + g2)
            nc.gpsimd.dma_start(out=tBc[0:G1N - 1, :], in_=tbf_gd[1][1:G1N, :])
            nc.gpsimd.dma_start(out=tBc[G1N - 1:cfg.WB_ROWS, :], in_=tbf_gd[2][:])
            for k in range(len(callsB)):
                emit_call(callsB[k], 1)
                if AOFF + k < len(callsA):
                    emit_call(callsA[AOFF + k], 0)
            flush_pending()

        # ---- cross-core pool reduction: AllGather + local sum ----
        poolsb = work.tile([NG, 65], F32, tag="poolsb")
        nc.vector.tensor_copy(poolsb[:], pool_ps[:])
        poolg = work.tile([NG, 65], F32, tag="poolg")
        if not collectives:
            nc.vector.tensor_copy(poolg[:], poolsb[:])
        else:
            nc.sync.dma_start(out=pool_in_d[:], in_=poolsb[:])
            nc.gpsimd.collective_compute(
                "AllGather", AL.bypass,
                replica_groups=[list(range(C))],
                ins=[pool_in_d[:]],
                outs=[pool_ag_d[:]],
            )
            pall = work.tile([NG, C, 65], F32, tag="pall")
            nc.sync.dma_start(
                out=pall[:], in_=pool_ag_d[:].rearrange("(c g) j -> g c j", g=NG))
            nc.vector.tensor_copy(poolg[:], pall[:, 0, :])
            for cc in range(1, C):
                nc.vector.tensor_add(poolg[:], poolg[:], pall[:, cc, :])
        # gfeat = pool / max(counts, 1)
        cnt = work.tile([NG, 1], F32, tag="cnt")
        nc.vector.tensor_scalar_max(cnt[:], poolg[:, 64:65], 1.0)
        nc.vector.reciprocal(cnt[:], cnt[:])
        gfeat = work.tile([NG, 64], F32, tag="gfeat")
        nc.vector.tensor_scalar_mul(gfeat[:], poolg[:, :64], cnt[:])
        # ---- decoder ----
        gfT_ps = pp.tile([64, NG], F32, tag="ps", space="PSUM")
        nc.tensor.transpose(out=gfT_ps[:], in_=gfeat[:], identity=ident[:NG, :NG])
        gfT = work.tile([64, NG], F32, tag="gfT")
        nc.vector.tensor_copy(gfT[:], gfT_ps[:])
        o1_ps = pp.tile([NG, 32], F32, tag="ps", space="PSUM")
        nc.tensor.matmul(out=o1_ps[:], lhsT=gfT[:], rhs=decw0[:],
                         start=True, stop=True, skip_group_check=True)
        o1 = work.tile([NG, 32], F32, tag="o1")
        nc.vector.tensor_add(o1[:], o1_ps[:], decb0[:])
        nc.scalar.activation(o1[:], o1[:], AF.Relu)
        o1T_ps = pp.tile([32, NG], F32, tag="ps", space="PSUM")
        nc.tensor.transpose(out=o1T_ps[:], in_=o1[:], identity=ident[:NG, :NG])
        o1T = work.tile([32, NG], F32, tag="o1T")
        nc.vector.tensor_copy(o1T[:], o1T_ps[:])
        o2_ps = pp.tile([NG, 1], F32, tag="ps", space="PSUM")
        nc.tensor.matmul(out=o2_ps[:], lhsT=o1T[:], rhs=decw1[:],
                         start=True, stop=True, skip_group_check=True)
        o2 = work.tile([NG, 1], F32, tag="o2")
        nc.vector.tensor_add(o2[:], o2_ps[:], decb1[:])
        nc.sync.dma_start(out=out_d[:], in_=o2[:])

    nc.compile()
    return nc


_COMPILED = {}


def kernel(**inputs):
    """Full-input entry point: shards across 8 NeuronCores internally."""
    from concourse.bass_utils import run_bass_kernel_spmd

    cfg = CFG_FULL
    edge_index = np.asarray(inputs["edge_index"])
    batch = np.asarray(inputs["batch"])
    sched, data = preprocess(cfg, edge_index, batch)
    key = (sched["T16"], sched["SA"], sched["SB"],
           sched["DA"].tobytes(), sched["DB"].tobytes())
    if key not in _COMPILED:
        _COMPILED.clear()
        _COMPILED[key] = build(cfg, sched)
    nc = _COMPILED[key]
    in_maps = per_core_inputs(cfg, sched, data, inputs)
    res = run_bass_kernel_spmd(nc, in_maps, list(range(cfg.C)), trace=False)
    out = np.asarray(res.results[0]["out"])[:, 0].astype(np.float32)
    return out


# revision 6
# speedup vs baseline: 3.0000x; 1.0500x over previous
"""Sharded DenseGNN Bass kernel for 8 TRN2 NeuronCores (overlapped fp8 collectives).

Design:
  - Nodes partitioned across 8 cores (6250/core), degree-sorted into 49
    tiles of 128 (tile 31 holds only 106 real nodes + 22 dummy slots).
  - Tiles are split into 3 groups (17/15/17 tiles) whose table regions
    align exactly with the two int16 gather windows:
      g0 rows [0,17408)  g1 rows [17408,32768)  g2 rows [32768,50176)
      window A = [0,32768) = g0+g1   window B = [17409,50176) = g1+g2
  - Per GCN layer: per group, compute hw = (h @ W') * dinv, DMA the slab
    chunk out, AllGather it (fp8 e4m3) into tbf_g, then cast-DMA into the f32
    gather table (window views A/B share the g1 region).  Window-A gathers depend
    only on {AG0, AG1}; window-B gathers only on {AG1, AG2}.  The next
    layer's matmuls and AllGathers are emitted inside the current B-phase
    (one gather-call lag) so collectives overlap gather DMA; the encoder +
    pooling are emitted inside layer 2's B-phase for the same reason.
    The table is double-buffered across layers.
  - Self-loop contribution is NOT gathered: msgs are initialised with the
    local dinv*hw value (Act engine) during the matmul phase.
  - Per-graph mean pooling via one-hot matmul, cross-core AllGather of the
    tiny [8, 65] pool partials, decoder computed redundantly on every core.

Structural preprocessing (index manipulation only) happens on the host;
the device computes dinv = rsqrt(deg) from host-counted integer degrees.
"""
import sys
import types

sys.path.insert(0, "/opt/trn_rl_repo")
if "antenv.axon_hooks" not in sys.modules:
    try:
        import antenv  # noqa: F401
        _m = types.ModuleType("antenv.axon_hooks")
        _m.get_axon_ntff_profile_hook = lambda: None
        sys.modules["antenv.axon_hooks"] = _m
    except ImportError:
        pass

import numpy as np

from concourse import bacc, bass, mybir, tile
from concourse.masks import make_identity

F32 = mybir.dt.float32
BF16 = mybir.dt.bfloat16
TDT = mybir.dt.float8e4  # collective transport dtype
I32 = mybir.dt.int32
I16 = mybir.dt.int16
NG = 8
BN_EPS = 1e-5


class Config:
    def __init__(self, n_nodes=50000, n_cores=8, max_idxs_per_call=12288):
        self.N = n_nodes
        self.C = n_cores
        self.NPC = n_nodes // n_cores          # 6250
        self.TPC = 49
        self.SPC = self.TPC * 128              # 6272
        self.TBL = self.C * self.SPC           # 50176
        # tile groups aligned with the two int16 windows
        self.GT = [(0, 17), (17, 32), (32, 49)]
        self.GS = [2176, 1920, 2176]           # per-core slots per group
        self.SOFF = [0, 2176, 4096]
        self.GOFF = [0, 17408, 32768, 50176]   # table row offsets
        self.DUMMY_TILE = 31
        self.DUMMY_P0 = 106                    # partitions >= this are dummy
        self.WINB = self.TBL - 32767           # 17409
        self.WA_ROWS = 32768
        self.WB_ROWS = 32767
        # pad rows point at the (always zero) core-0 dummy row in g1
        self.PAD_ROW = self.GOFF[1] + 1898     # 19306
        self.PAD_A = self.PAD_ROW              # < 32768
        self.PAD_B = self.PAD_ROW - self.WINB  # 11897
        assert self.PAD_A < self.WA_ROWS
        assert 0 <= self.PAD_B < self.WB_ROWS
        self.MAX_IDXS = max_idxs_per_call


CFG_FULL = Config()


def preprocess(cfg, edge_index, batch):
    """Build the SPMD-uniform schedule + per-core index data."""
    src_g = np.asarray(edge_index[0], dtype=np.int64)
    dst_g = np.asarray(edge_index[1], dtype=np.int64)
    N, C, SPC, TPC = cfg.N, cfg.C, cfg.SPC, cfg.TPC

    deg_in = np.bincount(dst_g, minlength=N).astype(np.int64)
    order_g = np.argsort(-deg_in, kind="stable")

    GS = np.array(cfg.GS)
    SOFF = np.array(cfg.SOFF)
    GOFF = np.array(cfg.GOFF[:3])
    slot_group = np.zeros(SPC, dtype=np.int64)
    for g in range(3):
        slot_group[cfg.SOFF[g]:cfg.SOFF[g] + cfg.GS[g]] = g

    perm = np.full((C, SPC), -1, dtype=np.int64)
    core_of = np.empty(N, dtype=np.int64)
    slot_of = np.empty(N, dtype=np.int64)
    pos = 0
    for k in range(TPC):
        bw = cfg.DUMMY_P0 if k == cfg.DUMMY_TILE else 128
        blk = order_g[pos:pos + C * bw]
        pos += C * bw
        for c in range(C):
            nodes = blk[c * bw:(c + 1) * bw]
            perm[c, k * 128:k * 128 + bw] = nodes
            core_of[nodes] = c
            slot_of[nodes] = k * 128 + np.arange(bw)
    assert pos == N

    sg = slot_group[slot_of]
    gpos = GOFF[sg] + core_of * GS[sg] + (slot_of - SOFF[sg])

    sgp = gpos[src_g]
    dst_core = core_of[dst_g]
    dst_slot = slot_of[dst_g]

    # classify for window split: 0=mustA, 1=flex, 2=mustB
    cls = np.ones(src_g.shape, dtype=np.int8)
    cls[sgp < cfg.WINB] = 0
    cls[sgp > 32767] = 2

    per_core = []
    dA_all = np.zeros((C, SPC), dtype=np.int64)
    dB_all = np.zeros((C, SPC), dtype=np.int64)
    for c in range(C):
        m = dst_core == c
        e_sgp = sgp[m]
        e_cls = cls[m]
        e_slot = dst_slot[m]
        order = np.lexsort((e_cls, e_slot))
        e_sgp = e_sgp[order]
        e_cls = e_cls[order]
        e_slot = e_slot[order]
        counts = np.bincount(e_slot, minlength=SPC)
        offs = np.concatenate([[0], np.cumsum(counts)])
        nA_must = np.bincount(e_slot[e_cls == 0], minlength=SPC)
        nflex = np.bincount(e_slot[e_cls == 1], minlength=SPC)
        need_a = np.clip((counts + 1) // 2 - nA_must, 0, nflex)
        dA = nA_must + need_a
        dB = counts - dA
        dA_all[c] = dA
        dB_all[c] = dB
        per_core.append(dict(e_sgp=e_sgp, offs=offs, dA=dA, dB=dB))

    # cross-core uniform per-tile slot widths
    DA = np.maximum(dA_all.reshape(C, TPC, 128).max(axis=(0, 2)), 1)
    DB = np.maximum(dB_all.reshape(C, TPC, 128).max(axis=(0, 2)), 1)

    # per-core padded slot tables [128, sum(D)] int32 (window-local indices)
    SA, SB = int(DA.sum()), int(DB.sum())
    offA = np.concatenate([[0], np.cumsum(DA)]).astype(np.int64)
    offB = np.concatenate([[0], np.cumsum(DB)]).astype(np.int64)
    intA = np.full((C, 128, SA), cfg.PAD_A, dtype=np.int32)
    intB = np.full((C, 128, SB), cfg.PAD_B, dtype=np.int32)
    for c in range(C):
        pc = per_core[c]
        e_sgp, offs, dA, dB = pc["e_sgp"], pc["offs"], pc["dA"], pc["dB"]
        node_of_edge = np.repeat(np.arange(SPC), np.diff(offs))
        rank = np.arange(e_sgp.size) - offs[node_of_edge]
        isA = rank < dA[node_of_edge]
        t_of_node = node_of_edge // 128
        p_of_node = node_of_edge % 128
        ra = rank[isA]
        intA[c, p_of_node[isA], offA[t_of_node[isA]] + ra] = e_sgp[isA]
        rb = rank[~isA] - dA[node_of_edge[~isA]]
        intB[c, p_of_node[~isA], offB[t_of_node[~isA]] + rb] = (
            e_sgp[~isA] - cfg.WINB)

    # gather-call schedule (uniform): batch consecutive tiles per window
    calls = []
    idx_off = 0
    for win, D, off in ((0, DA, offA), (1, DB, offB)):
        t = 0
        while t < TPC:
            t0, nidx, chunks = t, 0, 0
            while t < TPC:
                d = int(D[t])
                if nidx + d * 128 > cfg.MAX_IDXS and t > t0:
                    break
                chunks += d
                nidx += d * 128
                t += 1
            calls.append(dict(win=win, t0=t0, ntiles=t - t0, idx_off=idx_off,
                              nidx=nidx, chunks=chunks,
                              tile_D=[int(D[tt]) for tt in range(t0, t)]))
            idx_off += nidx
    total_idx = idx_off
    assert total_idx % 128 == 0
    T16 = total_idx // 16

    # per-core wrapped int16 idx lists
    idx16 = np.empty((C, 128, T16), dtype=np.int16)
    for c in range(C):
        flat = np.empty(total_idx, dtype=np.int16)
        for call in calls:
            win = call["win"]
            tbl = intA[c] if win == 0 else intB[c]
            off = offA if win == 0 else offB
            p = call["idx_off"]
            for k, tt in enumerate(range(call["t0"], call["t0"] + call["ntiles"])):
                d = call["tile_D"][k]
                blk = tbl[:, off[tt]:off[tt] + d]      # [128, d]
                flat[p:p + d * 128] = blk.T.reshape(-1).astype(np.int16)
                p += d * 128
            assert p == call["idx_off"] + call["nidx"]
        wrapped = flat.reshape(-1, 16).T               # [16, T16]
        idx16[c] = np.tile(wrapped, (8, 1))

    # per-core degree (incl. self loop) in tile-major layout, 1 on dummies
    degp = np.empty((C, 128, TPC), dtype=np.int32)
    for c in range(C):
        nodes = perm[c]
        dv = np.ones(SPC, dtype=np.int64)
        valid = nodes >= 0
        dv[valid] = deg_in[nodes[valid]] + 1
        degp[c] = dv.reshape(TPC, 128).T

    sched = dict(cfg=cfg, DA=DA, DB=DB, SA=SA, SB=SB,
                 offA=offA, offB=offB, calls=calls, T16=T16,
                 perm=perm, gpos=gpos)
    data = dict(idx16=idx16, degp=degp)
    return sched, data


def per_core_inputs(cfg, sched, data, inputs):
    """Build in_maps for run_bass_kernel_spmd."""
    x = np.asarray(inputs["x"], dtype=np.float32)
    batch = np.asarray(inputs["batch"], dtype=np.int64)
    perm = sched["perm"]
    C, SPC, TPC = cfg.C, cfg.SPC, cfg.TPC
    FIN = x.shape[1]

    common = {}
    for i in range(3):
        common[f"W{i}"] = np.ascontiguousarray(inputs[f"W{i}"], dtype=np.float32)
        for k in ("b", "g", "beta", "rm", "rv"):
            common[f"{k}{i}"] = np.ascontiguousarray(
                np.asarray(inputs[f"{k}{i}"], dtype=np.float32).reshape(-1, 1))
    common["enc_w0"] = np.ascontiguousarray(inputs["enc_w0"], dtype=np.float32)
    common["enc_w1"] = np.ascontiguousarray(inputs["enc_w1"], dtype=np.float32)
    common["dec_w0"] = np.ascontiguousarray(inputs["dec_w0"], dtype=np.float32)
    common["dec_w1"] = np.ascontiguousarray(inputs["dec_w1"], dtype=np.float32)
    for k in ("enc_b0", "enc_b1", "dec_b0", "dec_b1"):
        common[k] = np.ascontiguousarray(
            np.asarray(inputs[k], dtype=np.float32).reshape(-1, 1))

    in_maps = []
    for c in range(C):
        nodes = perm[c]                                # [SPC] with -1 dummies
        valid = nodes >= 0
        xv = np.zeros((SPC, FIN), dtype=np.float32)
        xv[valid] = x[nodes[valid]]
        xp = xv.reshape(TPC, 128, FIN).transpose(1, 0, 2).reshape(128, -1)
        bp = np.full((SPC,), NG, dtype=np.int64)
        bp[valid] = batch[nodes[valid]]
        bsb = bp.reshape(TPC, 128).T.astype(np.int32)  # [128, TPC]
        m = dict(common)
        m["xp"] = np.ascontiguousarray(xp)
        m["batchp"] = np.ascontiguousarray(bsb)
        m["idx16"] = np.ascontiguousarray(data["idx16"][c])
        m["degp"] = np.ascontiguousarray(data["degp"][c])
        in_maps.append(m)
    return in_maps


def build(cfg, sched, fin=3, collectives=True):
    C, TPC, SPC, TBL, T16 = cfg.C, cfg.TPC, cfg.SPC, cfg.TBL, sched["T16"]
    calls = sched["calls"]
    AL = mybir.AluOpType
    AF = mybir.ActivationFunctionType

    nc = bacc.Bacc(None, target_bir_lowering=False, debug=False, num_devices=C)

    # ---- kernel I/O ----
    xp_d = nc.dram_tensor("xp", [128, TPC * fin], F32, kind="ExternalInput")
    batch_d = nc.dram_tensor("batchp", [128, TPC], I32, kind="ExternalInput")
    idx16_d = nc.dram_tensor("idx16", [128, T16], I16, kind="ExternalInput")
    degp_d = nc.dram_tensor("degp", [128, TPC], I32, kind="ExternalInput")
    lp = []
    cins = [fin, 64, 128]
    for i in range(3):
        lp.append({
            "W": nc.dram_tensor(f"W{i}", [cins[i], 64], F32, kind="ExternalInput"),
            "b": nc.dram_tensor(f"b{i}", [64, 1], F32, kind="ExternalInput"),
            "g": nc.dram_tensor(f"g{i}", [64, 1], F32, kind="ExternalInput"),
            "beta": nc.dram_tensor(f"beta{i}", [64, 1], F32, kind="ExternalInput"),
            "rm": nc.dram_tensor(f"rm{i}", [64, 1], F32, kind="ExternalInput"),
            "rv": nc.dram_tensor(f"rv{i}", [64, 1], F32, kind="ExternalInput"),
        })
    encw0_d = nc.dram_tensor("enc_w0", [256, 128], F32, kind="ExternalInput")
    encb0_d = nc.dram_tensor("enc_b0", [128, 1], F32, kind="ExternalInput")
    encw1_d = nc.dram_tensor("enc_w1", [128, 64], F32, kind="ExternalInput")
    encb1_d = nc.dram_tensor("enc_b1", [64, 1], F32, kind="ExternalInput")
    decw0_d = nc.dram_tensor("dec_w0", [64, 32], F32, kind="ExternalInput")
    decb0_d = nc.dram_tensor("dec_b0", [32, 1], F32, kind="ExternalInput")
    decw1_d = nc.dram_tensor("dec_w1", [32, 1], F32, kind="ExternalInput")
    decb1_d = nc.dram_tensor("dec_b1", [1, 1], F32, kind="ExternalInput")
    out_d = nc.dram_tensor("out", [NG, 1], F32, kind="ExternalOutput")

    # ---- internal DRAM ----
    GS = cfg.GS
    slab_gd = [nc.dram_tensor(f"slab{g}", [GS[g], 64], TDT) for g in range(3)]
    tbf_gd = [nc.dram_tensor(f"tbf{g}", [C * GS[g], 64], TDT,
                             addr_space="Shared") for g in range(3)]
    tab_d = [nc.dram_tensor(f"tab{p}", [TBL, 64], F32) for p in range(2)]
    pool_in_d = nc.dram_tensor("pool_in", [NG, 65], F32)
    pool_ag_d = nc.dram_tensor("pool_ag", [C * NG, 65], F32, addr_space="Shared")

    G1N = cfg.GOFF[2] - cfg.GOFF[1]            # 15360 rows in g1

    from contextlib import ExitStack
    with tile.TileContext(nc) as tc, ExitStack() as es:
        const = es.enter_context(tc.tile_pool(name="const", bufs=1))
        work = es.enter_context(tc.tile_pool(name="work", bufs=3))
        gpool = es.enter_context(tc.tile_pool(name="gath", bufs=2))
        pp = es.enter_context(tc.tile_pool(name="ps", bufs=3, space="PSUM"))
        pp2 = es.enter_context(tc.tile_pool(name="ps2", bufs=4, space="PSUM"))
        ppacc = es.enter_context(tc.tile_pool(name="psacc", bufs=1, space="PSUM"))

        ident = const.tile([128, 128], F32, tag="ident")
        make_identity(nc, ident[:])
        ones_row = const.tile([1, 128], F32, tag="ones_row")
        nc.vector.memset(ones_row[:], 1.0)
        ones_col = const.tile([128, 1], F32, tag="ones_col")
        nc.vector.memset(ones_col[:], 1.0)
        iota8_i = const.tile([128, TPC * NG], I32, tag="iota8i")
        nc.gpsimd.iota(iota8_i[:], pattern=[[0, TPC], [1, NG]], base=0,
                       channel_multiplier=0)
        iota8 = const.tile([128, TPC * NG], F32, tag="iota8")
        nc.vector.tensor_copy(iota8[:], iota8_i[:])

        hfull = const.tile([128, TPC * 256], F32, tag="hfull")
        msgsb = const.tile([128, TPC * 64], F32, tag="msgsb")
        slabsb = const.tile([128, TPC * 64], TDT, tag="slabsb")
        dinv = const.tile([128, TPC], F32, tag="dinv")
        xsb = const.tile([128, TPC * fin], F32, tag="xsb")
        nc.sync.dma_start(out=xsb[:], in_=xp_d[:])
        batchsb = const.tile([128, TPC], I32, tag="batchsb")
        nc.sync.dma_start(out=batchsb[:], in_=batch_d[:])
        # one-hot graph membership for ALL tiles in one op: [128, TPC*NG]
        batchf = const.tile([128, TPC], F32, tag="batchf")
        nc.vector.tensor_copy(batchf[:], batchsb[:])
        oh_all = const.tile([128, TPC * NG], F32, tag="oh_all")
        nc.vector.tensor_tensor(
            out=oh_all[:].rearrange("p (t o) -> p t o", o=NG),
            in0=batchf[:].rearrange("p (t o) -> p t o", o=1).to_broadcast(
                [128, TPC, NG]),
            in1=iota8[:].rearrange("p (t o) -> p t o", o=NG),
            op=AL.is_equal)
        idx16sb = const.tile([128, T16], I16, tag="idx16sb")
        nc.sync.dma_start(out=idx16sb[:], in_=idx16_d[:])

        # ---- dinv = rsqrt(deg) from host-counted degrees ----
        degsb = work.tile([128, TPC], I32, tag="degsb")
        nc.sync.dma_start(out=degsb[:], in_=degp_d[:])
        nc.vector.tensor_copy(dinv[:], degsb[:])
        nc.scalar.sqrt(dinv[:], dinv[:])
        nc.vector.reciprocal(dinv[:], dinv[:])
        # zero dinv on the dummy partitions of the dummy tile so those slab
        # rows (pad-gather targets) are always zero
        dt_, dp0 = cfg.DUMMY_TILE, cfg.DUMMY_P0
        pidx_i = work.tile([128, 1], I32, tag="pidxi")
        nc.gpsimd.iota(pidx_i[:], pattern=[[1, 1]], base=0, channel_multiplier=1)
        pmask = work.tile([128, 1], F32, tag="pmask")
        nc.vector.tensor_copy(pmask[:], pidx_i[:])
        nc.vector.tensor_scalar(pmask[:], pmask[:], float(dp0), None, AL.is_lt)
        nc.vector.tensor_mul(dinv[:, dt_:dt_ + 1], dinv[:, dt_:dt_ + 1], pmask[:])

        # ---- BN fold: W' = W * s, bias'' = (b - rm) * s + beta ----
        wps = [None, None, None]
        bbcs = [None, None, None]

        def emit_bn_fold(i):
            cin = cins[i]
            g64 = work.tile([64, 1], F32, tag="p64")
            rv64 = work.tile([64, 1], F32, tag="p64b")
            s64 = const.tile([64, 1], F32, tag=f"s64_{i}")
            nc.sync.dma_start(out=rv64[:], in_=lp[i]["rv"][:])
            nc.vector.tensor_scalar_add(rv64[:], rv64[:], BN_EPS)
            nc.scalar.sqrt(rv64[:], rv64[:])
            nc.vector.reciprocal(rv64[:], rv64[:])
            nc.sync.dma_start(out=g64[:], in_=lp[i]["g"][:])
            nc.vector.tensor_mul(s64[:], g64[:], rv64[:])
            b64 = work.tile([64, 1], F32, tag="p64")
            rm64 = work.tile([64, 1], F32, tag="p64b")
            bb64 = const.tile([64, 1], F32, tag=f"bb64_{i}")
            nc.sync.dma_start(out=b64[:], in_=lp[i]["b"][:])
            nc.sync.dma_start(out=rm64[:], in_=lp[i]["rm"][:])
            nc.vector.tensor_sub(bb64[:], b64[:], rm64[:])
            nc.vector.tensor_mul(bb64[:], bb64[:], s64[:])
            be64 = work.tile([64, 1], F32, tag="p64")
            nc.sync.dma_start(out=be64[:], in_=lp[i]["beta"][:])
            nc.vector.tensor_add(bb64[:], bb64[:], be64[:])
            srow_ps = pp.tile([1, 64], F32, tag="ps", space="PSUM")
            nc.tensor.transpose(out=srow_ps[:], in_=s64[:], identity=ident[:64, :64])
            srow = work.tile([1, 64], F32, tag="row64")
            nc.vector.tensor_copy(srow[:], srow_ps[:])
            sbc_ps = pp.tile([128, 64], F32, tag="ps", space="PSUM")
            nc.tensor.matmul(out=sbc_ps[:], lhsT=ones_row[:1, :], rhs=srow[:],
                             start=True, stop=True)
            sbc = work.tile([128, 64], F32, tag="sbc")
            nc.vector.tensor_copy(sbc[:], sbc_ps[:])
            brow_ps = pp.tile([1, 64], F32, tag="ps", space="PSUM")
            nc.tensor.transpose(out=brow_ps[:], in_=bb64[:], identity=ident[:64, :64])
            brow = work.tile([1, 64], F32, tag="row64")
            nc.vector.tensor_copy(brow[:], brow_ps[:])
            bbc_ps = pp.tile([128, 64], F32, tag="ps", space="PSUM")
            nc.tensor.matmul(out=bbc_ps[:], lhsT=ones_row[:1, :], rhs=brow[:],
                             start=True, stop=True)
            bbc = const.tile([128, 64], F32, tag=f"bbc_{i}")
            nc.vector.tensor_copy(bbc[:], bbc_ps[:])
            bbcs[i] = bbc
            wraw = work.tile([cin, 64], F32, tag="wraw")
            nc.sync.dma_start(out=wraw[:], in_=lp[i]["W"][:])
            wp = const.tile([cin, 64], F32, tag=f"wp_{i}")
            nc.vector.tensor_mul(wp[:], wraw[:], sbc[:cin, :])
            wps[i] = wp

        in_off = {0: None, 1: 0, 2: 64}
        wr_off = {0: 0, 1: 128, 2: 192}

        def emit_mm_tile(li, t):
            cin = cins[li]
            if li == 0:
                hin = xsb[:, t * fin:(t + 1) * fin]
            else:
                o = t * 256 + in_off[li]
                hin = hfull[:, o:o + cin]
            tp_ps = pp.tile([cin, 128], F32, tag="ps", space="PSUM")
            nc.tensor.transpose(out=tp_ps[:], in_=hin, identity=ident[:])
            hT = work.tile([cin, 128], F32, tag="hT")
            nc.scalar.copy(hT[:], tp_ps[:])
            mm_ps = pp.tile([128, 64], F32, tag="ps", space="PSUM")
            nc.tensor.matmul(out=mm_ps[:], lhsT=hT[:], rhs=wps[li][:],
                             start=True, stop=True)
            dv = dinv[:, t:t + 1]
            nc.scalar.activation(slabsb[:, t * 64:(t + 1) * 64], mm_ps[:],
                                 AF.Copy, scale=dv)
            # msgs initialised with the self-loop term dinv*hw
            nc.scalar.activation(msgsb[:, t * 64:(t + 1) * 64], mm_ps[:],
                                 AF.Copy, scale=dv)

        def emit_post_group(li, g):
            t0g, t1g = cfg.GT[g]
            nc.sync.dma_start(
                out=slab_gd[g][:].rearrange("(t p) f -> p t f", p=128),
                in_=slabsb[:, t0g * 64:t1g * 64].rearrange(
                    "p (t f) -> p t f", f=64))
            if collectives:
                nc.gpsimd.collective_compute(
                    "AllGather", AL.bypass,
                    replica_groups=[list(range(C))],
                    ins=[slab_gd[g][:]],
                    outs=[tbf_gd[g][:]],
                )
            else:
                nc.sync.dma_start(out=tbf_gd[g][0:GS[g], :], in_=slab_gd[g][:])

        # encoder weights + broadcast biases (emitted in preamble)
        def emit_encdec_loads():
            nc.sync.dma_start(out=encw01[:], in_=encw0_d[0:128, :])
            nc.sync.dma_start(out=encw2[:], in_=encw0_d[128:192, :])
            nc.sync.dma_start(out=encw3[:], in_=encw0_d[192:256, :])
            nc.sync.dma_start(out=encw1[:], in_=encw1_d[:])
            nc.sync.dma_start(out=decw0[:], in_=decw0_d[:])
            nc.sync.dma_start(out=decw1[:], in_=decw1_d[:])

        encw01 = const.tile([128, 128], F32, tag="encw01")
        encw2 = const.tile([64, 128], F32, tag="encw2")
        encw3 = const.tile([64, 128], F32, tag="encw3")
        encw1 = const.tile([128, 64], F32, tag="encw1")
        decw0 = const.tile([64, 32], F32, tag="decw0")
        decw1 = const.tile([32, 1], F32, tag="decw1")
        e1part = const.tile([128, TPC * 128], F32, tag="e1part")

        def bcast_bias(d_param, flen, parts, tag):
            """Returns (broadcast [parts, flen], row [1, flen])."""
            v = work.tile([flen, 1], F32, tag="pbias")
            nc.sync.dma_start(out=v[:], in_=d_param[:])
            r_ps = pp.tile([1, flen], F32, tag="ps", space="PSUM")
            nc.tensor.transpose(out=r_ps[:], in_=v[:], identity=ident[:flen, :flen])
            r = const.tile([1, flen], F32, tag=f"{tag}_row")
            nc.vector.tensor_copy(r[:], r_ps[:])
            b_ps = pp.tile([parts, flen], F32, tag="ps", space="PSUM")
            nc.tensor.matmul(out=b_ps[:], lhsT=ones_row[:1, :parts], rhs=r[:],
                             start=True, stop=True)
            b = const.tile([parts, flen], F32, tag=tag)
            nc.vector.tensor_copy(b[:], b_ps[:])
            return b, r

        # ---- encoder + pooling, per tile (split across layer-2 A/B phases) ----
        pool_ps = ppacc.tile([NG, 65], F32, tag="pool", space="PSUM")

        def emit_enc_partial(t):
            # e1part = encb0 + [h1 h1]^T w[0:128] + h2^T w[128:192]
            # (h1/h2 are layer-0/1 outputs; available during layer 2's A phase)
            h2 = hfull[:, t * 256:(t + 1) * 256]
            tpa_ps = pp2.tile([128, 128], F32, tag="ps2", space="PSUM")
            nc.tensor.transpose(out=tpa_ps[:], in_=h2[:, 0:128], identity=ident[:])
            hTa = work.tile([128, 128], F32, tag="hT2")
            nc.scalar.copy(hTa[:], tpa_ps[:])
            tpb_ps = pp2.tile([64, 128], F32, tag="ps2", space="PSUM")
            nc.tensor.transpose(out=tpb_ps[:], in_=h2[:, 128:192], identity=ident[:])
            hTb = work.tile([64, 128], F32, tag="hTb")
            nc.scalar.copy(hTb[:], tpb_ps[:])
            e1p_ps = pp.tile([128, 128], F32, tag="ps", space="PSUM")
            nc.tensor.matmul(out=e1p_ps[:], lhsT=ones_row[:1, :], rhs=encb0_row[:],
                             start=True, stop=False, skip_group_check=True)
            nc.tensor.matmul(out=e1p_ps[:], lhsT=hTa[:], rhs=encw01[:],
                             start=False, stop=False, skip_group_check=True)
            nc.tensor.matmul(out=e1p_ps[:], lhsT=hTb[:], rhs=encw2[:],
                             start=False, stop=True, skip_group_check=True)
            nc.scalar.copy(e1part[:, t * 128:(t + 1) * 128], e1p_ps[:])

        def emit_encoder_tile(t):
            # finish e1 with the h3 contribution, then e2 + pooling
            h2 = hfull[:, t * 256:(t + 1) * 256]
            tpc_ps = pp2.tile([64, 128], F32, tag="ps2", space="PSUM")
            nc.tensor.transpose(out=tpc_ps[:], in_=h2[:, 192:256], identity=ident[:])
            hTc = work.tile([64, 128], F32, tag="hTb")
            nc.scalar.copy(hTc[:], tpc_ps[:])
            e1_ps = pp.tile([128, 128], F32, tag="ps", space="PSUM")
            nc.tensor.matmul(out=e1_ps[:], lhsT=ident[:],
                             rhs=e1part[:, t * 128:(t + 1) * 128],
                             start=True, stop=False, skip_group_check=True)
            nc.tensor.matmul(out=e1_ps[:], lhsT=hTc[:], rhs=encw3[:],
                             start=False, stop=True, skip_group_check=True)
            e1 = work.tile([128, 128], F32, tag="e1")
            nc.scalar.activation(e1[:], e1_ps[:], AF.Relu)
            tp2_ps = pp2.tile([128, 128], F32, tag="ps2", space="PSUM")
            nc.tensor.transpose(out=tp2_ps[:], in_=e1[:], identity=ident[:])
            e1T = work.tile([128, 128], F32, tag="hT2")
            nc.scalar.copy(e1T[:], tp2_ps[:])
            e2_ps = pp.tile([128, 64], F32, tag="ps", space="PSUM")
            nc.tensor.matmul(out=e2_ps[:], lhsT=ones_row[:1, :], rhs=encb1_row[:],
                             start=True, stop=False, skip_group_check=True)
            nc.tensor.matmul(out=e2_ps[:], lhsT=e1T[:], rhs=encw1[:],
                             start=False, stop=True, skip_group_check=True)
            e2 = work.tile([128, 65], F32, tag="e2")
            nc.scalar.activation(e2[:, :64], e2_ps[:], AF.Relu)
            nc.vector.tensor_copy(e2[:, 64:65], ones_col[:])
            nc.tensor.matmul(out=pool_ps[:, :65],
                             lhsT=oh_all[:, t * NG:(t + 1) * NG], rhs=e2[:],
                             start=(t == 0), stop=(t == TPC - 1),
                             skip_group_check=True)

        # =========================== emission ===========================
        emit_bn_fold(0)
        for t in range(*cfg.GT[0]):
            emit_mm_tile(0, t)
        emit_post_group(0, 0)
        emit_bn_fold(1)
        emit_bn_fold(2)
        for t in range(*cfg.GT[1]):
            emit_mm_tile(0, t)
        emit_post_group(0, 1)
        emit_encdec_loads()
        encb0, encb0_row = bcast_bias(encb0_d, 128, 128, "encb0")
        encb1, encb1_row = bcast_bias(encb1_d, 64, 128, "encb1")
        decb0, _ = bcast_bias(decb0_d, 32, NG, "decb0")
        decb1, _ = bcast_bias(decb1_d, 1, NG, "decb1")
        for t in range(*cfg.GT[2]):
            emit_mm_tile(0, t)
        emit_post_group(0, 2)

        callsA = [c for c in calls if c["win"] == 0]
        callsB = [c for c in calls if c["win"] == 1]
        group_end = {cfg.GT[g][1] - 1: g for g in range(3)}

        AOFF = 3  # A-calls emitted ahead of the interleaved B stream

        for li in range(3):
            tab = tab_d[li % 2]
            tAc = tab[0:cfg.WA_ROWS, :]
            tBc = tab[cfg.WINB:TBL, :]
            # upconverts needed by the A window (g0 + g1)
            nc.gpsimd.dma_start(out=tab[0:cfg.GOFF[1], :], in_=tbf_gd[0][:])
            nc.gpsimd.dma_start(out=tab[cfg.GOFF[1]:cfg.GOFF[2], :],
                                in_=tbf_gd[1][:])
            pending = []

            def flush_pending():
                while pending:
                    emit_post_group(*pending.pop(0))

            a_covered = -1  # max tile whose A contribution has been emitted

            def emit_call(call, win):
                nonlocal a_covered
                chunks, nidx = call["chunks"], call["nidx"]
                buf = gpool.tile([128, chunks, 64], F32, tag="gbuf")
                in_view = tAc if win == 0 else tBc
                c0 = call["idx_off"] // 16
                nc.gpsimd.dma_gather(
                    out_ap=buf[:],
                    in_ap=in_view,
                    idxs_ap=idx16sb[:, c0:c0 + nidx // 16],
                    num_idxs=nidx,
                    num_idxs_reg=nidx,
                    elem_size=64,
                    queue_num=0,
                    single_packet=False,
                )
                flush_pending()
                ci = 0
                for k in range(call["ntiles"]):
                    tt = call["t0"] + k
                    d = call["tile_D"][k]
                    seg = buf[:, ci:ci + d, :].rearrange("p d f -> p f d")
                    ci += d
                    mslice = msgsb[:, tt * 64:(tt + 1) * 64]
                    red = work.tile([128, 64], F32, tag="red")
                    nc.vector.tensor_reduce(
                        red[:], seg, axis=mybir.AxisListType.X, op=AL.add)
                    nc.vector.tensor_add(mslice, mslice, red[:])
                    if win == 0:
                        a_covered = tt
                        if li == 2:
                            emit_enc_partial(tt)
                    else:
                        assert tt <= a_covered, (tt, a_covered)
                        # epilogue: h = relu(msgs*dinv + bias)
                        w0 = wr_off[li]
                        hslice = hfull[:, tt * 256 + w0:tt * 256 + w0 + 64]
                        nc.vector.scalar_tensor_tensor(
                            out=hslice, in0=mslice, scalar=dinv[:, tt:tt + 1],
                            in1=bbcs[li][:], op0=AL.mult, op1=AL.add)
                        nc.scalar.activation(hslice, hslice, AF.Relu)
                        if li == 0:
                            nc.scalar.copy(
                                hfull[:, tt * 256 + 64:tt * 256 + 128], hslice)
                        if li < 2:
                            emit_mm_tile(li + 1, tt)
                            if tt in group_end:
                                pending.append((li + 1, group_end[tt]))
                        else:
                            emit_encoder_tile(tt)

            for call in callsA[:AOFF]:
                emit_call(call, 0)
            # upconvert needed by the B window (g2; g1 was written above)
            nc.gpsimd.dma_start(out=tab[cfg.GOFF[2]:TBL, :], in_=tbf_gd[2][:])
            for k in range(len(callsB)):
                emit_call(callsB[k], 1)
                if AOFF + k < len(callsA):
                    emit_call(callsA[AOFF + k], 0)
            flush_pending()

        # ---- cross-core pool reduction: AllGather + local sum ----
        poolsb = work.tile([NG, 65], F32, tag="poolsb")
        nc.vector.tensor_copy(poolsb[:], pool_ps[:])
        poolg = work.tile([NG, 65], F32, tag="poolg")
        if not collectives:
            nc.vector.tensor_copy(poolg[:], poolsb[:])
        else:
            nc.sync.dma_start(out=pool_in_d[:], in_=poolsb[:])
            nc.gpsimd.collective_compute(
                "AllGather", AL.bypass,
                replica_groups=[list(range(C))],
                ins=[pool_in_d[:]],
                outs=[pool_ag_d[:]],
            )
            pall = work.tile([NG, C, 65], F32, tag="pall")
            nc.sync.dma_start(
                out=pall[:], in_=pool_ag_d[:].rearrange("(c g) j -> g c j", g=NG))
            nc.vector.tensor_copy(poolg[:], pall[:, 0, :])
            for cc in range(1, C):
                nc.vector.tensor_add(poolg[:], poolg[:], pall[:, cc, :])
        # gfeat = pool / max(counts, 1)
        cnt = work.tile([NG, 1], F32, tag="cnt")
        nc.vector.tensor_scalar_max(cnt[:], poolg[:, 64:65], 1.0)
        nc.vector.reciprocal(cnt[:], cnt[:])
        gfeat = work.tile([NG, 64], F32, tag="gfeat")
        nc.vector.tensor_scalar_mul(gfeat[:], poolg[:, :64], cnt[:])
        # ---- decoder ----
        gfT_ps = pp.tile([64, NG], F32, tag="ps", space="PSUM")
        nc.tensor.transpose(out=gfT_ps[:], in_=gfeat[:], identity=ident[:NG, :NG])
        gfT = work.tile([64, NG], F32, tag="gfT")
        nc.vector.tensor_copy(gfT[:], gfT_ps[:])
        o1_ps = pp.tile([NG, 32], F32, tag="ps", space="PSUM")
        nc.tensor.matmul(out=o1_ps[:], lhsT=gfT[:], rhs=decw0[:],
                         start=True, stop=True, skip_group_check=True)
        o1 = work.tile([NG, 32], F32, tag="o1")
        nc.vector.tensor_add(o1[:], o1_ps[:], decb0[:])
        nc.scalar.activation(o1[:], o1[:], AF.Relu)
        o1T_ps = pp.tile([32, NG], F32, tag="ps", space="PSUM")
        nc.tensor.transpose(out=o1T_ps[:], in_=o1[:], identity=ident[:NG, :NG])
        o1T = work.tile([32, NG], F32, tag="o1T")
        nc.vector.tensor_copy(o1T[:], o1T_ps[:])
        o2_ps = pp.tile([NG, 1], F32, tag="ps", space="PSUM")
        nc.tensor.matmul(out=o2_ps[:], lhsT=o1T[:], rhs=decw1[:],
                         start=True, stop=True, skip_group_check=True)
        o2 = work.tile([NG, 1], F32, tag="o2")
        nc.vector.tensor_add(o2[:], o2_ps[:], decb1[:])
        nc.sync.dma_start(out=out_d[:], in_=o2[:])

    nc.compile()
    return nc


_COMPILED = {}


def kernel(**inputs):
    """Full-input entry point: shards across 8 NeuronCores internally."""
    from concourse.bass_utils import run_bass_kernel_spmd

    cfg = CFG_FULL
    edge_index = np.asarray(inputs["edge_index"])
    batch = np.asarray(inputs["batch"])
    sched, data = preprocess(cfg, edge_index, batch)
    key = (sched["T16"], sched["SA"], sched["SB"],
           sched["DA"].tobytes(), sched["DB"].tobytes())
    if key not in _COMPILED:
        _COMPILED.clear()
        _COMPILED[key] = build(cfg, sched)
    nc = _COMPILED[key]
    in_maps = per_core_inputs(cfg, sched, data, inputs)
    res = run_bass_kernel_spmd(nc, in_maps, list(range(cfg.C)), trace=False)
    out = np.asarray(res.results[0]["out"])[:, 0].astype(np.float32)
    return out
